# revision 48
# baseline (speedup 1.0000x reference)
"""Trainium2 Bass kernel for nn_AttentionBlock (B=4, L=S=1024, DIM=1024, NH=16).

Sharding: 8 cores = (batch b = core//2) x (head-half hh = core%2, 8 heads each).
Each core computes its batch's QKV projections restricted to its 512 feature
columns, attention for its 8 heads, and a partial output projection
(Wp row-slice); the host sums the two partials per batch.

Device layout is fully transposed ("T" = features/S on partitions) so no
on-device transposes are needed:
  qhT/khT (feat, L|S) from  Wslice.T @ xT ;  scoresT (S, L) = khT.T-slice @ qhT
  pos_bias+mask enter multiplicatively post-exp as M = mask*exp(pos_bias)
  precomputed on host (exp(s+b) = exp(s)*exp(b)); host pre-merges
  causal+row-fix into the mask. Softmax denominators ride a ones-column
  appended to V; normalization happens on the small (64, L) PV output via a
  rank-1 PE broadcast of 1/denom.

Single software-pipelined schedule (one flat 8-bank PSUM layout) so the tile
list-scheduler can overlap everything: QK proj for pair 0 runs up front, then
the attention unit loop (scores pair in concurrent 64-row PE groups -> merged
[128,1024] exp -> M-multiply -> PV) carries V-projection blocks and the next
pair's QK groups as PE filler; causally dead (key-chunk, query-half) tiles are
skipped in both compute and pb DMA. Output projection is lc-outer so its first
half overlaps the final pair's tail. Compute dtype bf16 (f32 PSUM
accumulation), f32 partial outputs.
"""
import contextlib
import ctypes
import sys
import types

import numpy as np
import ml_dtypes

bf16 = ml_dtypes.bfloat16

B, L, S, DIM, NH, DH = 4, 1024, 1024, 1024, 16, 64
NHC = 8           # heads per core
DIMC = 512        # feature columns per core
SCALE = 1.0 / np.sqrt(DH).astype(np.float32)

TRACE = False          # test.py flips this for profiling runs
TRACE_DIR = None
LAST_EXEC_NS = None


# ---------------------------------------------------------------- env setup
def _install_ntff_hook():
    if "antenv.axon_hooks" in sys.modules:
        return
    try:
        lib = ctypes.CDLL("/opt/axon/libaxon_pjrt.so")
        lib.axon_start_nrt_profile.argtypes = [
            ctypes.POINTER(ctypes.c_int64),
            ctypes.c_size_t,
        ]
        lib.axon_start_nrt_profile.restype = ctypes.c_int64
        lib.axon_stop_nrt_profile.argtypes = [ctypes.c_char_p]
        lib.axon_stop_nrt_profile.restype = ctypes.c_int64
    except OSError:
        return

    @contextlib.contextmanager
    def _hook(output_dir, device_ids):
        import jax

        jax.devices()
        if device_ids:
            ids = (ctypes.c_int64 * len(device_ids))(*device_ids)
            rc = lib.axon_start_nrt_profile(ids, len(device_ids))
        else:
            rc = lib.axon_start_nrt_profile(None, 0)
        if rc != 0:
            raise RuntimeError(f"axon_start_nrt_profile rc={rc}")
        try:
            yield
        finally:
            n = lib.axon_stop_nrt_profile(str(output_dir).encode())
            print(f"profile: {n} file(s) written to {output_dir}")

    mod = types.ModuleType("antenv.axon_hooks")
    mod.get_axon_ntff_profile_hook = lambda: _hook
    mod.set_axon_ntff_profile_hook = lambda h: None
    sys.modules["antenv.axon_hooks"] = mod


def _patch_tile_drain():
    from concourse import mybir
    from concourse.tile import TileContext, ScopedClock

    if getattr(TileContext, "_drain_split_patched", False):
        return

    def _drain_and_barrier(self, tick_clock, wait_clock):
        drain_inst = self.nc.sync.drain()
        wait_clock.add_sem_waits(
            drain_inst.ins, ScopedClock({None: tick_clock.global_clock})
        )
        waits = list(drain_inst.ins.sync_info.on_wait)
        if len(waits) > 1:
            drain_inst.ins.sync_info.on_wait = waits[:1]
            for w in waits[1:]:
                nop = self.nc.sync.nop()
                nop.ins.sync_info = mybir.SyncInfo(on_wait=[w], on_update=[])
        self.nc.all_engine_barrier()
        assert self.sems is not None
        popped = self.nc._tile_sem_poison_stack.pop()
        assert popped is self._sem_poison
        self.nc.clear_and_free_semaphores(list(self.sems.allocated().values()))
        self.nc.all_engine_barrier()

    TileContext._drain_and_barrier = _drain_and_barrier
    TileContext._drain_split_patched = True


def _split_multiwait_instructions(nc):
    """This container's walrus rejects >1 sync wait per instruction; hoist
    extras onto same-engine NOPs placed right before the instruction."""
    from concourse import mybir

    n_split = 0
    for fn in nc.m.functions:
        for bb in fn.blocks:
            out = []
            for inst in bb.instructions:
                si = inst.sync_info
                waits = list(si.on_wait) if si is not None else []
                if len(waits) > 1:
                    for w in waits[:-1]:
                        n_split += 1
                        out.append(
                            mybir.InstNoOp(
                                name=f"waitsplit-{n_split}-{inst.name}",
                                engine=inst.engine,
                                bass_nofuse=True,
                                sync_info=mybir.SyncInfo(on_wait=[w], on_update=[]),
                            )
                        )
                    si.on_wait = waits[-1:]
                out.append(inst)
            if n_split:
                bb.instructions = out


# ---------------------------------------------------------------- builder
_NC_CACHE = {}


def build_nc(use_bq=False, use_bk=False, use_bv=False, use_bp=False, live=None):
    if live is None:
        live = tuple(tuple(True for _ in range(2)) for _ in range(8))
    key = (use_bq, use_bk, use_bv, use_bp, live)
    if key in _NC_CACHE:
        return _NC_CACHE[key]
    _install_ntff_hook()
    _patch_tile_drain()
    import concourse.bass as bass
    import concourse.tile as tile
    from concourse import mybir

    dt = mybir.dt
    AF = mybir.ActivationFunctionType

    nc = bass.Bass("TRN2", target_bir_lowering=False, debug=False, num_devices=8)

    qT_d = nc.declare_dram_parameter("qT", (DIM, L), dt.bfloat16, isOutput=False)
    kT_d = nc.declare_dram_parameter("kT", (DIM, S), dt.bfloat16, isOutput=False)
    vT_d = nc.declare_dram_parameter("vT", (DIM, S), dt.bfloat16, isOutput=False)
    wq_d = nc.declare_dram_parameter("wq", (DIM, DIMC), dt.bfloat16, isOutput=False)
    wk_d = nc.declare_dram_parameter("wk", (DIM, DIMC), dt.bfloat16, isOutput=False)
    wv_d = nc.declare_dram_parameter("wv", (DIM, DIMC), dt.bfloat16, isOutput=False)
    wp_d = nc.declare_dram_parameter("wp", (DIMC, DIM), dt.bfloat16, isOutput=False)
    pb_d = nc.declare_dram_parameter("pbT", (4 * S, 2 * L), dt.bfloat16, isOutput=False)
    bq_d = nc.declare_dram_parameter("bq", (1, DIMC), dt.float32, isOutput=False)
    bk_d = nc.declare_dram_parameter("bk", (1, DIMC), dt.float32, isOutput=False)
    bv_d = nc.declare_dram_parameter("bv", (1, DIMC), dt.float32, isOutput=False)
    bp_d = nc.declare_dram_parameter("bp", (128, 8), dt.float32, isOutput=False)
    out_d = nc.declare_dram_parameter("out", (DIM, L), dt.float32, isOutput=True)

    with tile.TileContext(nc) as tc:
        with (
            tc.tile_pool(name="consts", bufs=1) as consts,
            tc.tile_pool(name="w", bufs=1) as wpool,
            tc.tile_pool(name="heads", bufs=1) as heads,
            tc.tile_pool(name="stage", bufs=1) as stage,
            tc.tile_pool(name="ostage", bufs=4) as ostage,
            tc.tile_pool(name="pb", bufs=2) as pbp,
            tc.tile_pool(name="attn", bufs=6) as attnp,
        ):
            ones_t = consts.tile([128, 64], dt.bfloat16)
            nc.gpsimd.memset(ones_t[:], 1.0)
            if use_bq:
                bq_t = consts.tile([1, DIMC], dt.float32)
                nc.sync.dma_start(bq_t[:], bq_d[:])
            if use_bk:
                bk_t = consts.tile([1, DIMC], dt.float32)
                nc.sync.dma_start(bk_t[:], bk_d[:])
            if use_bv:
                bv_t = consts.tile([1, DIMC], dt.float32)
                nc.sync.dma_start(bv_t[:], bv_d[:])
                ones_f = consts.tile([1, 128], dt.float32)
                nc.gpsimd.memset(ones_f[:], 1.0)
            if use_bq or use_bk:
                ones_r = consts.tile([1, 512], dt.float32)
                nc.gpsimd.memset(ones_r[:], 1.0)

            # big consolidated weight tiles: w*[:, dt*512 + p*128 + ...] = W[dt*128+p-row, col]
            wq_t = wpool.tile([128, 8 * DIMC], dt.bfloat16, name="wqb", tag="wqb")
            wk_t = wpool.tile([128, 8 * DIMC], dt.bfloat16, name="wkb", tag="wkb")
            wv_t = wpool.tile([128, 8 * DIMC], dt.bfloat16, name="wvb", tag="wvb")
            wp_t = wpool.tile([128, 4 * DIM], dt.bfloat16, name="wpb", tag="wpb")

            qh_t = [heads.tile([128, L], dt.bfloat16, name=f"qh{i}", tag=f"qh{i}") for i in range(4)]
            kh_t = [heads.tile([128, S], dt.bfloat16, name=f"kh{i}", tag=f"kh{i}") for i in range(4)]
            vh_t = [heads.tile([128, NHC * 65], dt.bfloat16, name=f"vh{i}", tag=f"vh{i}") for i in range(8)]
            oT_t = [heads.tile([128, L], dt.bfloat16, name=f"oT{i}", tag=f"oT{i}") for i in range(4)]

            def load_big(tile_ap, dram, rows, cols, chunks=1):
                # tile[:, a*cols + c] = dram[a*128 + p, c]
                n_a = rows // 128
                a_per = n_a // chunks
                for ch in range(chunks):
                    nc.sync.dma_start(
                        tile_ap[:, ch * a_per * cols : (ch + 1) * a_per * cols]
                        .rearrange("p (a c) -> p a c", c=cols),
                        dram[ch * a_per * 128 : (ch + 1) * a_per * 128, :]
                        .rearrange("(a p) c -> p a c", p=128),
                    )

            # ======= unified pipeline: one flat PSUM layout (8 banks total) =======
            # projps 2x[128,512] + scps 1x[128,1024] + pvps 3x[65,512] + prps 1
            with (
                tc.tile_pool(name="xT", bufs=1) as xTp,
                tc.tile_pool(name="projps", bufs=2, space="PSUM") as pps,
                tc.tile_pool(name="scps", bufs=1, space="PSUM") as scps,
                tc.tile_pool(name="pvps", bufs=3, space="PSUM") as pvps,
                tc.tile_pool(name="prps", bufs=1, space="PSUM") as prps,
            ):
                # DMA issue costs ~620ns/trigger on the issuing engine's queue,
                # so split streams across queues: q/k on sync, v + pair-0 pb on
                # the (startup-idle) scalar queue. Weights load in 2 big chunks.
                xq, xk, xv = [], [], []
                for x_l, nm, w_t, w_dram, x_dram, eng in (
                    (xq, "xq", wq_t, wq_d, qT_d, nc.sync),
                    (xv, "xv", wv_t, wv_d, vT_d, nc.scalar),
                    (xk, "xk", wk_t, wk_d, kT_d, nc.sync),
                ):
                    for ch in range(2):
                        eng.dma_start(
                            w_t[:, ch * 4 * 512 : (ch + 1) * 4 * 512]
                            .rearrange("p (a c) -> p a c", c=512),
                            w_dram[ch * 4 * 128 : (ch + 1) * 4 * 128, :]
                            .rearrange("(a p) c -> p a c", p=128),
                        )
                        for dtile in range(ch * 4, (ch + 1) * 4):
                            x_t = xTp.tile(
                                [128, 1024], dt.bfloat16, name=f"{nm}{dtile}", tag=f"{nm}{dtile}"
                            )
                            eng.dma_start(
                                x_t[:], x_dram[dtile * 128 : (dtile + 1) * 128, :]
                            )
                            x_l.append(x_t)

                def load_pb_lc(pb_t, p, lc2, eng):
                    # only live (key-chunk, query-half) blocks are ever read
                    for a in range(8):
                        if live[a][lc2]:
                            eng.dma_start(
                                pb_t[:, a * 2048 + lc2 * 1024 : a * 2048 + (lc2 + 1) * 1024],
                                pb_d[
                                    p * S + a * 128 : p * S + (a + 1) * 128,
                                    lc2 * 1024 : (lc2 + 1) * 1024,
                                ],
                            )

                pb0_t = pbp.tile([128, 16 * L], dt.bfloat16, name="pb_p0", tag="pb")
                load_pb_lc(pb0_t, 0, 0, nc.scalar)
                load_pb_lc(pb0_t, 0, 1, nc.scalar)

                def emit_qk_group(p, kind, lc):
                    lcs = slice(lc * 512, (lc + 1) * 512)
                    w_t, x_l, use_b, dst = (
                        (wq_t, xq, use_bq, qh_t)
                        if kind == "q"
                        else (wk_t, xk, use_bk, kh_t)
                    )
                    ps = pps.tile([128, 512], dt.float32, name=f"ps{kind}{p}_{lc}", tag="ps")
                    for dtile in range(8):
                        nc.tensor.matmul(
                            ps[:],
                            w_t[:, dtile * 512 + p * 128 : dtile * 512 + (p + 1) * 128],
                            x_l[dtile][:, lcs],
                            start=(dtile == 0),
                            stop=(dtile == 7) and not use_b,
                        )
                    if use_b:
                        b_t = bq_t if kind == "q" else bk_t
                        nc.tensor.matmul(
                            ps[:],
                            b_t[0:1, p * 128 : (p + 1) * 128],
                            ones_r[0:1, 0:512],
                            start=False, stop=True,
                        )
                    nc.vector.tensor_copy(dst[p][:, lcs], ps[:])

                def emit_v_block(st):
                    ps = pps.tile([128, 512], dt.float32, name=f"psv{st}", tag="ps")
                    for dtile in range(8):
                        nc.tensor.matmul(
                            ps[:],
                            xv[dtile][:, st * 128 : (st + 1) * 128],
                            wv_t[:, dtile * 512 : (dtile + 1) * 512],
                            start=(dtile == 0),
                            stop=(dtile == 7) and not use_bv,
                        )
                    if use_bv:
                        nc.tensor.matmul(
                            ps[:], ones_f[0:1, 0:128], bv_t[:], start=False, stop=True
                        )
                    nc.gpsimd.memset(vh_t[st][:], 1.0)
                    nc.scalar.copy(
                        vh_t[st].rearrange("p (h x) -> p h x", x=65)[:, :, 0:64],
                        ps.rearrange("p (h x) -> p h x", x=64),
                    )

                # head-pair 0 QK up front; everything else rides the unit loop
                for kind, lc in (("q", 0), ("k", 0), ("q", 1), ("k", 1)):
                    emit_qk_group(0, kind, lc)

                load_big(wp_t, wp_d, DIMC, DIM)
                bp_t = consts.tile([128, 8], dt.float32)
                if use_bp:
                    nc.sync.dma_start(bp_t[:], bp_d[:])

                LAG = 2
                NLAG = 1
                live_sts = {
                    lc: [st for st in range(8) if live[st][lc]] for lc in range(2)
                }
                first_live = {lc: live_sts[lc][0] for lc in range(2)}
                last_live = {lc: live_sts[lc][-1] for lc in range(2)}
                units = [
                    (p, st, lc)
                    for p in range(4)
                    for lc in range(2)
                    for st in range(8)
                    if live[st][lc]
                ]
                ats = {}
                pos = {}
                pbs = {}

                first_unit_of_p = {}
                for idx, (p, st, lc) in enumerate(units):
                    first_unit_of_p.setdefault(p, idx)
                pair_ps = sorted(first_unit_of_p)
                n_in_pair = {
                    p: (first_unit_of_p[pair_ps[k + 1]] if k + 1 < len(pair_ps) else len(units))
                    - first_unit_of_p[p]
                    for k, p in enumerate(pair_ps)
                }

                def load_pb(p):
                    pb_t = pbp.tile([128, 16 * L], dt.bfloat16, name=f"pb{p}", tag="pb")
                    load_pb_lc(pb_t, p, 0, nc.sync)
                    load_pb_lc(pb_t, p, 1, nc.sync)
                    pbs[p] = pb_t

                pbs[0] = pb0_t  # loaded with the startup DMA stream

                def emit_scores(i):
                    p, st, lc = units[i]
                    lcs = slice(lc * 512, (lc + 1) * 512)
                    if i == first_unit_of_p[p]:
                        k_next = pair_ps.index(p) + 1
                        if k_next < len(pair_ps):
                            load_pb(pair_ps[k_next])
                        for j in range(2):
                            h = 2 * p + j
                            for k in range(2):
                                pos[(h, k)] = pvps.tile(
                                    [65, 512], dt.float32, name=f"po{h}_{k}", tag="pv"
                                )
                    ps = scps.tile([128, 1024], dt.float32, name=f"sc{p}_{st}_{lc}", tag="sc")
                    for j in range(2):
                        jj = j * 64
                        nc.tensor.matmul(
                            ps[:, jj * 8 : jj * 8 + 512],
                            kh_t[p][jj : jj + 64, st * 128 : (st + 1) * 128],
                            qh_t[p][jj : jj + 64, lcs],
                            start=True, stop=True,
                            tile_position=(jj, 0),
                        )
                    at = attnp.tile([128, 1024], dt.bfloat16, name=f"at{p}_{st}_{lc}", tag="attn")
                    nc.scalar.activation(at[:], ps[:], AF.Exp)
                    nc.vector.tensor_mul(
                        at[:], at[:],
                        pbs[p][:, st * 2048 + lc * 1024 : st * 2048 + (lc + 1) * 1024],
                    )
                    ats[(p, st, lc)] = at

                def emit_pv(i):
                    p, st, lc = units[i]
                    at = ats.pop((p, st, lc))
                    for j in range(2):
                        h = 2 * p + j
                        nc.tensor.matmul(
                            pos[(h, lc)][:],
                            vh_t[st][:, h * 65 : h * 65 + 65],
                            at[:, j * 512 : (j + 1) * 512],
                            start=(st == first_live[lc]),
                            stop=(st == last_live[lc]),
                        )

                def emit_norm(p, lc):
                    lcs = slice(lc * 512, (lc + 1) * 512)
                    for j in range(2):
                        h = 2 * p + j
                        po = pos.pop((h, lc))
                        rec_bf = stage.tile(
                            [65, 512], dt.bfloat16, name=f"recbf{h}_{lc}", tag="recbf"
                        )
                        lnr = stage.tile([65, 512], dt.float32, name=f"lnr{h}_{lc}", tag="lnr")
                        nc.scalar.activation(lnr[64:65, :], po[64:65, :], AF.Ln)
                        nc.scalar.activation(
                            rec_bf[64:65, :], lnr[64:65, :], AF.Exp, scale=-1.0
                        )
                        pr = prps.tile([64, 512], dt.float32, name=f"pr{h}_{lc}", tag="pr")
                        nc.tensor.matmul(
                            pr[:],
                            ones_t[64:65, 0:64],
                            rec_bf[64:65, :],
                            start=True, stop=True,
                        )
                        pr_sb = stage.tile(
                            [64, 512], dt.float32, name=f"prsb{h}_{lc}", tag="prsb"
                        )
                        nc.vector.tensor_copy(pr_sb[:], pr[:])
                        nc.vector.tensor_mul(
                            oT_t[p][j * 64 : (j + 1) * 64, lcs], po[0:64, :], pr_sb[:]
                        )

                # PE filler work interleaved into the unit stream: V blocks and
                # next pair's QK groups run in attention-phase PE gaps
                filler = {}
                p0 = pair_ps[0]
                for st in range(8):
                    filler.setdefault(
                        first_unit_of_p[p0] + min(st, n_in_pair[p0] - 1), []
                    ).append(lambda st=st: emit_v_block(st))
                for k, p in enumerate(pair_ps[1:]):
                    src = pair_ps[k]
                    n_src = n_in_pair[src]
                    offs = [8, 9, 10, 11] if k == 0 else [0, 2, 4, 6]
                    for (kind, lc2), o in zip(
                        (("q", 0), ("k", 0), ("q", 1), ("k", 1)), offs
                    ):
                        filler.setdefault(
                            first_unit_of_p[src] + min(o, n_src - 1), []
                        ).append(
                            lambda p=p, kind=kind, lc2=lc2: emit_qk_group(p, kind, lc2)
                        )

                norm_due = {}
                n_units = len(units)
                for i in range(n_units + LAG + NLAG + 4):
                    if i in norm_due:
                        for p, lc in norm_due.pop(i):
                            emit_norm(p, lc)
                    if i < n_units:
                        emit_scores(i)
                    ipv = i - LAG
                    if 0 <= ipv < n_units:
                        emit_pv(ipv)
                        p, st, lc = units[ipv]
                        if st == last_live[lc]:
                            norm_due.setdefault(i + NLAG, []).append((p, lc))
                    for fn in filler.pop(i, []):
                        fn()
                assert not norm_due and not pos and not ats and not filler

                # ========== output projection (lc-outer so lc0 can start early) ==========
                for lc in range(2):
                    lcs = slice(lc * 512, (lc + 1) * 512)
                    for ot in range(8):
                        pf = pps.tile([128, 512], dt.float32, name=f"pf{ot}_{lc}", tag="ps")
                        for p4 in range(4):
                            nc.tensor.matmul(
                                pf[:],
                                wp_t[:, p4 * 1024 + ot * 128 : p4 * 1024 + (ot + 1) * 128],
                                oT_t[p4][:, lcs],
                                start=(p4 == 0),
                                stop=(p4 == 3),
                            )
                        f_sb = ostage.tile([128, 512], dt.float32, name=f"fsb{ot}_{lc}", tag="fsb")
                        if use_bp:
                            nc.scalar.activation(
                                f_sb[:], pf[:], AF.Identity, bias=bp_t[:, ot : ot + 1]
                            )
                        else:
                            nc.scalar.copy(f_sb[:], pf[:])
                        nc.sync.dma_start(
                            out_d[ot * 128 : (ot + 1) * 128, lcs], f_sb[:]
                        )

    _split_multiwait_instructions(nc)
    _NC_CACHE[key] = nc
    return nc


# ---------------------------------------------------------------- host side
def prep_inputs(inputs):
    """Shard + lay out the full inputs into 8 per-core input maps."""
    q = np.asarray(inputs["q"], np.float32)
    k = np.asarray(inputs["k"], np.float32)
    v = np.asarray(inputs["v"], np.float32)
    attn_mask = np.asarray(inputs["attn_mask"], bool)
    pos_bias = np.asarray(inputs["pos_bias"], np.float32)
    Wq = np.asarray(inputs["Wq"], np.float32)
    Wk = np.asarray(inputs["Wk"], np.float32)
    Wv = np.asarray(inputs["Wv"], np.float32)
    Wp = np.asarray(inputs["Wp"], np.float32)
    bq = np.asarray(inputs["bq"], np.float32)
    bk = np.asarray(inputs["bk"], np.float32)
    bv = np.asarray(inputs["bv"], np.float32)
    bp = np.asarray(inputs["bp"], np.float32)
    is_causal = int(np.asarray(inputs["is_causal"]))

    # effective mask: causal + row-any fix (matches the reference exactly)
    mask = attn_mask
    if is_causal:
        causal = np.tril(np.ones((L, L), bool))
        causal = np.pad(causal, ((0, 0), (S - L, 0)), constant_values=True)
        mask = mask & causal[None]
    row_any = mask.any(axis=-1, keepdims=True)
    mask = np.where(row_any, mask, True)  # (B, L, S)

    in_maps = []
    for core in range(8):
        b, hh = core // 2, core % 2
        c0 = hh * DIMC
        h0 = hh * NHC
        wq_c = (Wq[:, c0 : c0 + DIMC] * SCALE).astype(bf16)
        wk_c = Wk[:, c0 : c0 + DIMC].astype(bf16)
        wv_c = Wv[:, c0 : c0 + DIMC].astype(bf16)
        wp_c = Wp[c0 : c0 + DIMC, :].astype(bf16)
        # M = mask * exp(pos_bias), laid out per head-pair as
        # pbT[p*S + key, lc*1024 + j*512 + qc] = M[h=2p+j, query=lc*512+qc, key]
        pbT = np.empty((4 * S, 2 * L), bf16)
        mb = mask[b]  # (L, S)
        for pp in range(4):
            for j in range(2):
                E = np.exp(pos_bias[b, h0 + 2 * pp + j])  # (L, S)
                E *= mb
                for lc in range(2):
                    pbT[
                        pp * S : (pp + 1) * S,
                        lc * 1024 + j * 512 : lc * 1024 + (j + 1) * 512,
                    ] = E[lc * 512 : (lc + 1) * 512, :].T.astype(bf16)
        in_maps.append(
            dict(
                qT=q[b].T.astype(bf16),
                kT=k[b].T.astype(bf16),
                vT=v[b].T.astype(bf16),
                wq=np.ascontiguousarray(wq_c),
                wk=np.ascontiguousarray(wk_c),
                wv=np.ascontiguousarray(wv_c),
                wp=np.ascontiguousarray(wp_c),
                pbT=pbT,
                bq=np.ascontiguousarray((bq[c0 : c0 + DIMC] * SCALE)[None, :]),
                bk=np.ascontiguousarray(bk[c0 : c0 + DIMC][None, :]),
                bv=np.ascontiguousarray(bv[c0 : c0 + DIMC][None, :]),
                bp=(
                    np.ascontiguousarray(bp.reshape(8, 128).T)
                    if hh == 0
                    else np.zeros((128, 8), np.float32)
                ),
            )
        )
    # per-(S-tile, L-chunk) liveness: union of the effective mask over batches.
    # A tile that is all-masked for every batch contributes exactly zero
    # (mask multiply) so the shared SPMD program can skip it entirely.
    mt = mask.any(axis=0)  # (L, S) union over batches
    live = tuple(
        tuple(
            bool(mt[lc * 512 : (lc + 1) * 512, st * 128 : (st + 1) * 128].any())
            for lc in range(2)
        )
        for st in range(8)
    )
    return in_maps, live


def kernel(**inputs):
    global LAST_EXEC_NS
    from concourse.bass_utils import run_bass_kernel_spmd

    in_maps, live = prep_inputs(inputs)
    nc = build_nc(
        use_bq=bool(np.any(np.asarray(inputs["bq"]))),
        use_bk=bool(np.any(np.asarray(inputs["bk"]))),
        use_bv=bool(np.any(np.asarray(inputs["bv"]))),
        use_bp=bool(np.any(np.asarray(inputs["bp"]))),
        live=live,
    )
    kwargs = {}
    if TRACE and TRACE_DIR:
        kwargs["tmpdir"] = TRACE_DIR
    res = run_bass_kernel_spmd(
        nc, in_maps, core_ids=list(range(8)), trace=TRACE, **kwargs
    )
    LAST_EXEC_NS = res.exec_time_ns
    outs = res.results
    out = np.empty((B, L, DIM), np.float32)
    for b in range(B):
        out[b] = (outs[2 * b]["out"] + outs[2 * b + 1]["out"]).T
    return out



# revision 52
# speedup vs baseline: 1.0847x; 1.0847x over previous
"""Trainium2 Bass kernel for nn_AttentionBlock (B=4, L=S=1024, DIM=1024, NH=16).

Sharding: 8 cores = (batch b = core//2) x (head-half hh = core%2, 8 heads each).
Each core computes its batch's QKV projections restricted to its 512 feature
columns, attention for its 8 heads, and a partial output projection
(Wp row-slice); the host sums the two partials per batch.

Device layout is fully transposed ("T" = features/S on partitions) so no
on-device transposes are needed:
  qhT/khT (feat, L|S) from  Wslice.T @ xT ;  scoresT (S, L) = khT.T-slice @ qhT
  pos_bias+mask enter multiplicatively post-exp as M = mask*exp(pos_bias)
  precomputed on host (exp(s+b) = exp(s)*exp(b)); host pre-merges
  causal+row-fix into the mask. Softmax denominators ride a ones-column
  appended to V; normalization happens on the small (64, L) PV output via a
  rank-1 PE broadcast of 1/denom.

Single software-pipelined schedule (one flat 8-bank PSUM layout) so the tile
list-scheduler can overlap everything: QK proj for pair 0 runs up front, then
the attention unit loop (scores pair in concurrent 64-row PE groups -> merged
[128,1024] exp -> M-multiply -> PV) carries V-projection blocks and the next
pair's QK groups as PE filler; causally dead (key-chunk, query-half) tiles are
skipped in both compute and pb DMA. Output projection is lc-outer so its first
half overlaps the final pair's tail. Compute dtype bf16 (f32 PSUM
accumulation), f32 partial outputs.
"""
import contextlib
import ctypes
import sys
import types

import numpy as np
import ml_dtypes

bf16 = ml_dtypes.bfloat16

B, L, S, DIM, NH, DH = 4, 1024, 1024, 1024, 16, 64
NHC = 8           # heads per core
DIMC = 512        # feature columns per core
SCALE = 1.0 / np.sqrt(DH).astype(np.float32)

TRACE = False          # test.py flips this for profiling runs
TRACE_DIR = None
LAST_EXEC_NS = None


# ---------------------------------------------------------------- env setup
def _install_ntff_hook():
    if "antenv.axon_hooks" in sys.modules:
        return
    try:
        lib = ctypes.CDLL("/opt/axon/libaxon_pjrt.so")
        lib.axon_start_nrt_profile.argtypes = [
            ctypes.POINTER(ctypes.c_int64),
            ctypes.c_size_t,
        ]
        lib.axon_start_nrt_profile.restype = ctypes.c_int64
        lib.axon_stop_nrt_profile.argtypes = [ctypes.c_char_p]
        lib.axon_stop_nrt_profile.restype = ctypes.c_int64
    except OSError:
        return

    @contextlib.contextmanager
    def _hook(output_dir, device_ids):
        import jax

        jax.devices()
        if device_ids:
            ids = (ctypes.c_int64 * len(device_ids))(*device_ids)
            rc = lib.axon_start_nrt_profile(ids, len(device_ids))
        else:
            rc = lib.axon_start_nrt_profile(None, 0)
        if rc != 0:
            raise RuntimeError(f"axon_start_nrt_profile rc={rc}")
        try:
            yield
        finally:
            n = lib.axon_stop_nrt_profile(str(output_dir).encode())
            print(f"profile: {n} file(s) written to {output_dir}")

    mod = types.ModuleType("antenv.axon_hooks")
    mod.get_axon_ntff_profile_hook = lambda: _hook
    mod.set_axon_ntff_profile_hook = lambda h: None
    sys.modules["antenv.axon_hooks"] = mod


def _patch_tile_drain():
    from concourse import mybir
    from concourse.tile import TileContext, ScopedClock

    if getattr(TileContext, "_drain_split_patched", False):
        return

    def _drain_and_barrier(self, tick_clock, wait_clock):
        drain_inst = self.nc.sync.drain()
        wait_clock.add_sem_waits(
            drain_inst.ins, ScopedClock({None: tick_clock.global_clock})
        )
        waits = list(drain_inst.ins.sync_info.on_wait)
        if len(waits) > 1:
            drain_inst.ins.sync_info.on_wait = waits[:1]
            for w in waits[1:]:
                nop = self.nc.sync.nop()
                nop.ins.sync_info = mybir.SyncInfo(on_wait=[w], on_update=[])
        self.nc.all_engine_barrier()
        assert self.sems is not None
        popped = self.nc._tile_sem_poison_stack.pop()
        assert popped is self._sem_poison
        self.nc.clear_and_free_semaphores(list(self.sems.allocated().values()))
        self.nc.all_engine_barrier()

    TileContext._drain_and_barrier = _drain_and_barrier
    TileContext._drain_split_patched = True


def _split_multiwait_instructions(nc):
    """This container's walrus rejects >1 sync wait per instruction; hoist
    extras onto same-engine NOPs placed right before the instruction."""
    from concourse import mybir

    n_split = 0
    for fn in nc.m.functions:
        for bb in fn.blocks:
            out = []
            for inst in bb.instructions:
                si = inst.sync_info
                waits = list(si.on_wait) if si is not None else []
                if len(waits) > 1:
                    for w in waits[:-1]:
                        n_split += 1
                        out.append(
                            mybir.InstNoOp(
                                name=f"waitsplit-{n_split}-{inst.name}",
                                engine=inst.engine,
                                bass_nofuse=True,
                                sync_info=mybir.SyncInfo(on_wait=[w], on_update=[]),
                            )
                        )
                    si.on_wait = waits[-1:]
                out.append(inst)
            if n_split:
                bb.instructions = out


# ---------------------------------------------------------------- builder
_NC_CACHE = {}


def build_nc(use_bq=False, use_bk=False, use_bv=False, use_bp=False, live=None):
    if live is None:
        live = tuple(tuple(True for _ in range(2)) for _ in range(8))
    key = (use_bq, use_bk, use_bv, use_bp, live)
    if key in _NC_CACHE:
        return _NC_CACHE[key]
    _install_ntff_hook()
    _patch_tile_drain()
    import concourse.bass as bass
    import concourse.tile as tile
    from concourse import mybir

    dt = mybir.dt
    AF = mybir.ActivationFunctionType

    nc = bass.Bass("TRN2", target_bir_lowering=False, debug=False, num_devices=8)

    qT_d = nc.declare_dram_parameter("qT", (DIM, L), dt.bfloat16, isOutput=False)
    kT_d = nc.declare_dram_parameter("kT", (DIM, S), dt.bfloat16, isOutput=False)
    vT_d = nc.declare_dram_parameter("vT", (DIM, S), dt.bfloat16, isOutput=False)
    wq_d = nc.declare_dram_parameter("wq", (DIM, DIMC), dt.bfloat16, isOutput=False)
    wk_d = nc.declare_dram_parameter("wk", (DIM, DIMC), dt.bfloat16, isOutput=False)
    wv_d = nc.declare_dram_parameter("wv", (DIM, DIMC), dt.bfloat16, isOutput=False)
    wp_d = nc.declare_dram_parameter("wp", (DIMC, DIM), dt.bfloat16, isOutput=False)
    pb_d = nc.declare_dram_parameter("pbT", (4 * S, 2 * L), dt.bfloat16, isOutput=False)
    bq_d = nc.declare_dram_parameter("bq", (1, DIMC), dt.float32, isOutput=False)
    bk_d = nc.declare_dram_parameter("bk", (1, DIMC), dt.float32, isOutput=False)
    bv_d = nc.declare_dram_parameter("bv", (1, DIMC), dt.float32, isOutput=False)
    bp_d = nc.declare_dram_parameter("bp", (128, 8), dt.float32, isOutput=False)
    out_d = nc.declare_dram_parameter("out", (DIM, L), dt.float32, isOutput=True)

    with tile.TileContext(nc) as tc:
        with (
            tc.tile_pool(name="consts", bufs=1) as consts,
            tc.tile_pool(name="w", bufs=1) as wpool,
            tc.tile_pool(name="heads", bufs=1) as heads,
            tc.tile_pool(name="stage", bufs=1) as stage,
            tc.tile_pool(name="ostage", bufs=4) as ostage,
            tc.tile_pool(name="pb", bufs=2) as pbp,
            tc.tile_pool(name="attn", bufs=6) as attnp,
        ):
            ones_t = consts.tile([128, 64], dt.bfloat16)
            nc.gpsimd.memset(ones_t[:], 1.0)
            if use_bq:
                bq_t = consts.tile([1, DIMC], dt.float32)
                nc.sync.dma_start(bq_t[:], bq_d[:])
            if use_bk:
                bk_t = consts.tile([1, DIMC], dt.float32)
                nc.sync.dma_start(bk_t[:], bk_d[:])
            if use_bv:
                bv_t = consts.tile([1, DIMC], dt.float32)
                nc.sync.dma_start(bv_t[:], bv_d[:])
                ones_f = consts.tile([1, 128], dt.float32)
                nc.gpsimd.memset(ones_f[:], 1.0)
            if use_bq or use_bk:
                ones_r = consts.tile([1, 512], dt.float32)
                nc.gpsimd.memset(ones_r[:], 1.0)

            # big consolidated weight tiles: w*[:, dt*512 + p*128 + ...] = W[dt*128+p-row, col]
            wq_t = wpool.tile([128, 8 * DIMC], dt.bfloat16, name="wqb", tag="wqb")
            wk_t = wpool.tile([128, 8 * DIMC], dt.bfloat16, name="wkb", tag="wkb")
            wv_t = wpool.tile([128, 8 * DIMC], dt.bfloat16, name="wvb", tag="wvb")
            wp_t = wpool.tile([128, 4 * DIM], dt.bfloat16, name="wpb", tag="wpb")

            qh_t = [heads.tile([128, L], dt.bfloat16, name=f"qh{i}", tag=f"qh{i}") for i in range(4)]
            kh_t = [heads.tile([128, S], dt.bfloat16, name=f"kh{i}", tag=f"kh{i}") for i in range(4)]
            vh_t = [heads.tile([128, NHC * 65], dt.bfloat16, name=f"vh{i}", tag=f"vh{i}") for i in range(8)]
            oT_t = [heads.tile([128, L], dt.bfloat16, name=f"oT{i}", tag=f"oT{i}") for i in range(4)]

            def load_big(tile_ap, dram, rows, cols, chunks=1):
                # tile[:, a*cols + c] = dram[a*128 + p, c]
                n_a = rows // 128
                a_per = n_a // chunks
                for ch in range(chunks):
                    nc.sync.dma_start(
                        tile_ap[:, ch * a_per * cols : (ch + 1) * a_per * cols]
                        .rearrange("p (a c) -> p a c", c=cols),
                        dram[ch * a_per * 128 : (ch + 1) * a_per * 128, :]
                        .rearrange("(a p) c -> p a c", p=128),
                    )

            # ======= unified pipeline: one flat PSUM layout (8 banks total) =======
            # projps 2x[128,512] + scps 1x[128,1024] + pvps 3x[65,512] + prps 1
            with (
                tc.tile_pool(name="xT", bufs=1) as xTp,
                tc.tile_pool(name="projps", bufs=2, space="PSUM") as pps,
                tc.tile_pool(name="scps", bufs=1, space="PSUM") as scps,
                tc.tile_pool(name="pvps", bufs=3, space="PSUM") as pvps,
                tc.tile_pool(name="prps", bufs=1, space="PSUM") as prps,
            ):
                # input DMA, dtile-interleaved so accumulation starts on chunk 0
                xq, xk, xv = [], [], []
                for x_l, nm, w_t, w_dram, x_dram in (
                    (xq, "xq", wq_t, wq_d, qT_d),
                    (xk, "xk", wk_t, wk_d, kT_d),
                    (xv, "xv", wv_t, wv_d, vT_d),
                ):
                    for dtile in range(8):
                        nc.sync.dma_start(
                            w_t[:, dtile * 512 : (dtile + 1) * 512],
                            w_dram[dtile * 128 : (dtile + 1) * 128, :],
                        )
                        x_t = xTp.tile(
                            [128, 1024], dt.bfloat16, name=f"{nm}{dtile}", tag=f"{nm}{dtile}"
                        )
                        nc.sync.dma_start(x_t[:], x_dram[dtile * 128 : (dtile + 1) * 128, :])
                        x_l.append(x_t)

                def emit_qk_group(p, kind, lc):
                    lcs = slice(lc * 512, (lc + 1) * 512)
                    w_t, x_l, use_b, dst = (
                        (wq_t, xq, use_bq, qh_t)
                        if kind == "q"
                        else (wk_t, xk, use_bk, kh_t)
                    )
                    ps = pps.tile([128, 512], dt.float32, name=f"ps{kind}{p}_{lc}", tag="ps")
                    for dtile in range(8):
                        nc.tensor.matmul(
                            ps[:],
                            w_t[:, dtile * 512 + p * 128 : dtile * 512 + (p + 1) * 128],
                            x_l[dtile][:, lcs],
                            start=(dtile == 0),
                            stop=(dtile == 7) and not use_b,
                        )
                    if use_b:
                        b_t = bq_t if kind == "q" else bk_t
                        nc.tensor.matmul(
                            ps[:],
                            b_t[0:1, p * 128 : (p + 1) * 128],
                            ones_r[0:1, 0:512],
                            start=False, stop=True,
                        )
                    nc.vector.tensor_copy(dst[p][:, lcs], ps[:])

                def emit_v_block(st):
                    ps = pps.tile([128, 512], dt.float32, name=f"psv{st}", tag="ps")
                    for dtile in range(8):
                        nc.tensor.matmul(
                            ps[:],
                            xv[dtile][:, st * 128 : (st + 1) * 128],
                            wv_t[:, dtile * 512 : (dtile + 1) * 512],
                            start=(dtile == 0),
                            stop=(dtile == 7) and not use_bv,
                        )
                    if use_bv:
                        nc.tensor.matmul(
                            ps[:], ones_f[0:1, 0:128], bv_t[:], start=False, stop=True
                        )
                    nc.gpsimd.memset(vh_t[st][:], 1.0)
                    nc.scalar.copy(
                        vh_t[st].rearrange("p (h x) -> p h x", x=65)[:, :, 0:64],
                        ps.rearrange("p (h x) -> p h x", x=64),
                    )

                # head-pair 0 QK up front; everything else rides the unit loop
                for kind, lc in (("q", 0), ("k", 0), ("q", 1), ("k", 1)):
                    emit_qk_group(0, kind, lc)

                load_big(wp_t, wp_d, DIMC, DIM)
                bp_t = consts.tile([128, 8], dt.float32)
                if use_bp:
                    nc.sync.dma_start(bp_t[:], bp_d[:])

                LAG = 2
                NLAG = 1
                live_sts = {
                    lc: [st for st in range(8) if live[st][lc]] for lc in range(2)
                }
                first_live = {lc: live_sts[lc][0] for lc in range(2)}
                last_live = {lc: live_sts[lc][-1] for lc in range(2)}
                units = [
                    (p, st, lc)
                    for p in range(4)
                    for st in range(8)
                    for lc in range(2)
                    if live[st][lc]
                ]
                ats = {}
                pos = {}
                pbs = {}

                first_unit_of_p = {}
                for idx, (p, st, lc) in enumerate(units):
                    first_unit_of_p.setdefault(p, idx)
                pair_ps = sorted(first_unit_of_p)
                n_in_pair = {
                    p: (first_unit_of_p[pair_ps[k + 1]] if k + 1 < len(pair_ps) else len(units))
                    - first_unit_of_p[p]
                    for k, p in enumerate(pair_ps)
                }

                def load_pb(p, eng):
                    # only live (key-chunk, query-half) blocks are ever read;
                    # cols are (st, lc, j, 512) so a (st, lc) block is contiguous
                    pb_t = pbp.tile([128, 16 * L], dt.bfloat16, name=f"pb{p}", tag="pb")
                    for a in range(8):
                        lcs_live = [lc2 for lc2 in range(2) if live[a][lc2]]
                        if not lcs_live:
                            continue
                        c0 = lcs_live[0] * 1024
                        c1 = (lcs_live[-1] + 1) * 1024
                        eng.dma_start(
                            pb_t[:, a * 2048 + c0 : a * 2048 + c1],
                            pb_d[p * S + a * 128 : p * S + (a + 1) * 128, c0:c1],
                        )
                    pbs[p] = pb_t

                # pair-0 pb rides the startup-idle scalar queue so its 12
                # trigger slots (~620ns each) stay off sync's critical path
                load_pb(pair_ps[0], nc.scalar)

                def emit_scores(i):
                    p, st, lc = units[i]
                    lcs = slice(lc * 512, (lc + 1) * 512)
                    if i == first_unit_of_p[p]:
                        k_next = pair_ps.index(p) + 1
                        if k_next < len(pair_ps):
                            load_pb(pair_ps[k_next], nc.sync)
                        for j in range(2):
                            h = 2 * p + j
                            for k in range(2):
                                pos[(h, k)] = pvps.tile(
                                    [65, 512], dt.float32, name=f"po{h}_{k}", tag="pv"
                                )
                    ps = scps.tile([128, 1024], dt.float32, name=f"sc{p}_{st}_{lc}", tag="sc")
                    for j in range(2):
                        jj = j * 64
                        nc.tensor.matmul(
                            ps[:, jj * 8 : jj * 8 + 512],
                            kh_t[p][jj : jj + 64, st * 128 : (st + 1) * 128],
                            qh_t[p][jj : jj + 64, lcs],
                            start=True, stop=True,
                            tile_position=(jj, 0),
                        )
                    at = attnp.tile([128, 1024], dt.bfloat16, name=f"at{p}_{st}_{lc}", tag="attn")
                    nc.scalar.activation(at[:], ps[:], AF.Exp)
                    nc.vector.tensor_mul(
                        at[:], at[:],
                        pbs[p][:, st * 2048 + lc * 1024 : st * 2048 + (lc + 1) * 1024],
                    )
                    ats[(p, st, lc)] = at

                def emit_pv(i):
                    p, st, lc = units[i]
                    at = ats.pop((p, st, lc))
                    for j in range(2):
                        h = 2 * p + j
                        nc.tensor.matmul(
                            pos[(h, lc)][:],
                            vh_t[st][:, h * 65 : h * 65 + 65],
                            at[:, j * 512 : (j + 1) * 512],
                            start=(st == first_live[lc]),
                            stop=(st == last_live[lc]),
                        )

                def emit_norm(p, lc):
                    lcs = slice(lc * 512, (lc + 1) * 512)
                    for j in range(2):
                        h = 2 * p + j
                        po = pos.pop((h, lc))
                        rec_bf = stage.tile(
                            [65, 512], dt.bfloat16, name=f"recbf{h}_{lc}", tag="recbf"
                        )
                        lnr = stage.tile([65, 512], dt.float32, name=f"lnr{h}_{lc}", tag="lnr")
                        nc.scalar.activation(lnr[64:65, :], po[64:65, :], AF.Ln)
                        nc.scalar.activation(
                            rec_bf[64:65, :], lnr[64:65, :], AF.Exp, scale=-1.0
                        )
                        pr = prps.tile([64, 512], dt.float32, name=f"pr{h}_{lc}", tag="pr")
                        nc.tensor.matmul(
                            pr[:],
                            ones_t[64:65, 0:64],
                            rec_bf[64:65, :],
                            start=True, stop=True,
                        )
                        pr_sb = stage.tile(
                            [64, 512], dt.float32, name=f"prsb{h}_{lc}", tag="prsb"
                        )
                        nc.vector.tensor_copy(pr_sb[:], pr[:])
                        nc.vector.tensor_mul(
                            oT_t[p][j * 64 : (j + 1) * 64, lcs], po[0:64, :], pr_sb[:]
                        )

                # PE filler work interleaved into the unit stream: V blocks and
                # next pair's QK groups run in attention-phase PE gaps
                filler = {}
                p0 = pair_ps[0]
                for st in range(8):
                    filler.setdefault(
                        first_unit_of_p[p0] + min(st, n_in_pair[p0] - 1), []
                    ).append(lambda st=st: emit_v_block(st))
                for k, p in enumerate(pair_ps[1:]):
                    src = pair_ps[k]
                    n_src = n_in_pair[src]
                    offs = [8, 9, 10, 11] if k == 0 else [0, 2, 4, 6]
                    for (kind, lc2), o in zip(
                        (("q", 0), ("k", 0), ("q", 1), ("k", 1)), offs
                    ):
                        filler.setdefault(
                            first_unit_of_p[src] + min(o, n_src - 1), []
                        ).append(
                            lambda p=p, kind=kind, lc2=lc2: emit_qk_group(p, kind, lc2)
                        )

                norm_due = {}
                n_units = len(units)
                for i in range(n_units + LAG + NLAG + 4):
                    if i in norm_due:
                        for p, lc in norm_due.pop(i):
                            emit_norm(p, lc)
                    if i < n_units:
                        emit_scores(i)
                    ipv = i - LAG
                    if 0 <= ipv < n_units:
                        emit_pv(ipv)
                        p, st, lc = units[ipv]
                        if st == last_live[lc]:
                            norm_due.setdefault(i + NLAG, []).append((p, lc))
                    for fn in filler.pop(i, []):
                        fn()
                assert not norm_due and not pos and not ats and not filler

                # ========== output projection (lc-outer so lc0 can start early) ==========
                for lc in range(2):
                    lcs = slice(lc * 512, (lc + 1) * 512)
                    for ot in range(8):
                        pf = pps.tile([128, 512], dt.float32, name=f"pf{ot}_{lc}", tag="ps")
                        for p4 in range(4):
                            nc.tensor.matmul(
                                pf[:],
                                wp_t[:, p4 * 1024 + ot * 128 : p4 * 1024 + (ot + 1) * 128],
                                oT_t[p4][:, lcs],
                                start=(p4 == 0),
                                stop=(p4 == 3),
                            )
                        f_sb = ostage.tile([128, 512], dt.float32, name=f"fsb{ot}_{lc}", tag="fsb")
                        if use_bp:
                            nc.scalar.activation(
                                f_sb[:], pf[:], AF.Identity, bias=bp_t[:, ot : ot + 1]
                            )
                        else:
                            nc.scalar.copy(f_sb[:], pf[:])
                        nc.sync.dma_start(
                            out_d[ot * 128 : (ot + 1) * 128, lcs], f_sb[:]
                        )

    _split_multiwait_instructions(nc)
    _NC_CACHE[key] = nc
    return nc


# ---------------------------------------------------------------- host side
def prep_inputs(inputs):
    """Shard + lay out the full inputs into 8 per-core input maps."""
    q = np.asarray(inputs["q"], np.float32)
    k = np.asarray(inputs["k"], np.float32)
    v = np.asarray(inputs["v"], np.float32)
    attn_mask = np.asarray(inputs["attn_mask"], bool)
    pos_bias = np.asarray(inputs["pos_bias"], np.float32)
    Wq = np.asarray(inputs["Wq"], np.float32)
    Wk = np.asarray(inputs["Wk"], np.float32)
    Wv = np.asarray(inputs["Wv"], np.float32)
    Wp = np.asarray(inputs["Wp"], np.float32)
    bq = np.asarray(inputs["bq"], np.float32)
    bk = np.asarray(inputs["bk"], np.float32)
    bv = np.asarray(inputs["bv"], np.float32)
    bp = np.asarray(inputs["bp"], np.float32)
    is_causal = int(np.asarray(inputs["is_causal"]))

    # effective mask: causal + row-any fix (matches the reference exactly)
    mask = attn_mask
    if is_causal:
        causal = np.tril(np.ones((L, L), bool))
        causal = np.pad(causal, ((0, 0), (S - L, 0)), constant_values=True)
        mask = mask & causal[None]
    row_any = mask.any(axis=-1, keepdims=True)
    mask = np.where(row_any, mask, True)  # (B, L, S)

    in_maps = []
    for core in range(8):
        b, hh = core // 2, core % 2
        c0 = hh * DIMC
        h0 = hh * NHC
        wq_c = (Wq[:, c0 : c0 + DIMC] * SCALE).astype(bf16)
        wk_c = Wk[:, c0 : c0 + DIMC].astype(bf16)
        wv_c = Wv[:, c0 : c0 + DIMC].astype(bf16)
        wp_c = Wp[c0 : c0 + DIMC, :].astype(bf16)
        # M = mask * exp(pos_bias), laid out per head-pair as
        # pbT[p*S + key, lc*1024 + j*512 + qc] = M[h=2p+j, query=lc*512+qc, key]
        pbT = np.empty((4 * S, 2 * L), bf16)
        mb = mask[b]  # (L, S)
        for pp in range(4):
            for j in range(2):
                E = np.exp(pos_bias[b, h0 + 2 * pp + j])  # (L, S)
                E *= mb
                for lc in range(2):
                    pbT[
                        pp * S : (pp + 1) * S,
                        lc * 1024 + j * 512 : lc * 1024 + (j + 1) * 512,
                    ] = E[lc * 512 : (lc + 1) * 512, :].T.astype(bf16)
        in_maps.append(
            dict(
                qT=q[b].T.astype(bf16),
                kT=k[b].T.astype(bf16),
                vT=v[b].T.astype(bf16),
                wq=np.ascontiguousarray(wq_c),
                wk=np.ascontiguousarray(wk_c),
                wv=np.ascontiguousarray(wv_c),
                wp=np.ascontiguousarray(wp_c),
                pbT=pbT,
                bq=np.ascontiguousarray((bq[c0 : c0 + DIMC] * SCALE)[None, :]),
                bk=np.ascontiguousarray(bk[c0 : c0 + DIMC][None, :]),
                bv=np.ascontiguousarray(bv[c0 : c0 + DIMC][None, :]),
                bp=(
                    np.ascontiguousarray(bp.reshape(8, 128).T)
                    if hh == 0
                    else np.zeros((128, 8), np.float32)
                ),
            )
        )
    # per-(S-tile, L-chunk) liveness: union of the effective mask over batches.
    # A tile that is all-masked for every batch contributes exactly zero
    # (mask multiply) so the shared SPMD program can skip it entirely.
    mt = mask.any(axis=0)  # (L, S) union over batches
    live = tuple(
        tuple(
            bool(mt[lc * 512 : (lc + 1) * 512, st * 128 : (st + 1) * 128].any())
            for lc in range(2)
        )
        for st in range(8)
    )
    return in_maps, live


def kernel(**inputs):
    global LAST_EXEC_NS
    from concourse.bass_utils import run_bass_kernel_spmd

    in_maps, live = prep_inputs(inputs)
    nc = build_nc(
        use_bq=bool(np.any(np.asarray(inputs["bq"]))),
        use_bk=bool(np.any(np.asarray(inputs["bk"]))),
        use_bv=bool(np.any(np.asarray(inputs["bv"]))),
        use_bp=bool(np.any(np.asarray(inputs["bp"]))),
        live=live,
    )
    kwargs = {}
    if TRACE and TRACE_DIR:
        kwargs["tmpdir"] = TRACE_DIR
    res = run_bass_kernel_spmd(
        nc, in_maps, core_ids=list(range(8)), trace=TRACE, **kwargs
    )
    LAST_EXEC_NS = res.exec_time_ns
    outs = res.results
    out = np.empty((B, L, DIM), np.float32)
    for b in range(B):
        out[b] = (outs[2 * b]["out"] + outs[2 * b + 1]["out"]).T
    return out



# revision 53
# speedup vs baseline: 1.1478x; 1.0582x over previous
"""Trainium2 Bass kernel for nn_AttentionBlock (B=4, L=S=1024, DIM=1024, NH=16).

Sharding: 8 cores = (batch b = core//2) x (head-half hh = core%2, 8 heads each).
Each core computes its batch's QKV projections restricted to its 512 feature
columns, attention for its 8 heads, and a partial output projection
(Wp row-slice); the host sums the two partials per batch.

Device layout is fully transposed ("T" = features/S on partitions) so no
on-device transposes are needed:
  qhT/khT (feat, L|S) from  Wslice.T @ xT ;  scoresT (S, L) = khT.T-slice @ qhT
  pos_bias+mask enter multiplicatively post-exp as M = mask*exp(pos_bias)
  precomputed on host (exp(s+b) = exp(s)*exp(b)); host pre-merges
  causal+row-fix into the mask. Softmax denominators ride a ones-column
  appended to V; normalization happens on the small (64, L) PV output via a
  rank-1 PE broadcast of 1/denom.

Single software-pipelined schedule (one flat 8-bank PSUM layout) so the tile
list-scheduler can overlap everything: QK proj for pair 0 runs up front, then
the attention unit loop (scores pair in concurrent 64-row PE groups -> merged
[128,1024] exp -> M-multiply -> PV) carries V-projection blocks and the next
pair's QK groups as PE filler; causally dead (key-chunk, query-half) tiles are
skipped in both compute and pb DMA. Output projection is lc-outer so its first
half overlaps the final pair's tail. Compute dtype bf16 (f32 PSUM
accumulation), f32 partial outputs.
"""
import contextlib
import ctypes
import sys
import types

import numpy as np
import ml_dtypes

bf16 = ml_dtypes.bfloat16

B, L, S, DIM, NH, DH = 4, 1024, 1024, 1024, 16, 64
NHC = 8           # heads per core
DIMC = 512        # feature columns per core
SCALE = 1.0 / np.sqrt(DH).astype(np.float32)

TRACE = False          # test.py flips this for profiling runs
TRACE_DIR = None
LAST_EXEC_NS = None


# ---------------------------------------------------------------- env setup
def _install_ntff_hook():
    if "antenv.axon_hooks" in sys.modules:
        return
    try:
        lib = ctypes.CDLL("/opt/axon/libaxon_pjrt.so")
        lib.axon_start_nrt_profile.argtypes = [
            ctypes.POINTER(ctypes.c_int64),
            ctypes.c_size_t,
        ]
        lib.axon_start_nrt_profile.restype = ctypes.c_int64
        lib.axon_stop_nrt_profile.argtypes = [ctypes.c_char_p]
        lib.axon_stop_nrt_profile.restype = ctypes.c_int64
    except OSError:
        return

    @contextlib.contextmanager
    def _hook(output_dir, device_ids):
        import jax

        jax.devices()
        if device_ids:
            ids = (ctypes.c_int64 * len(device_ids))(*device_ids)
            rc = lib.axon_start_nrt_profile(ids, len(device_ids))
        else:
            rc = lib.axon_start_nrt_profile(None, 0)
        if rc != 0:
            raise RuntimeError(f"axon_start_nrt_profile rc={rc}")
        try:
            yield
        finally:
            n = lib.axon_stop_nrt_profile(str(output_dir).encode())
            print(f"profile: {n} file(s) written to {output_dir}")

    mod = types.ModuleType("antenv.axon_hooks")
    mod.get_axon_ntff_profile_hook = lambda: _hook
    mod.set_axon_ntff_profile_hook = lambda h: None
    sys.modules["antenv.axon_hooks"] = mod


def _patch_tile_drain():
    from concourse import mybir
    from concourse.tile import TileContext, ScopedClock

    if getattr(TileContext, "_drain_split_patched", False):
        return

    def _drain_and_barrier(self, tick_clock, wait_clock):
        drain_inst = self.nc.sync.drain()
        wait_clock.add_sem_waits(
            drain_inst.ins, ScopedClock({None: tick_clock.global_clock})
        )
        waits = list(drain_inst.ins.sync_info.on_wait)
        if len(waits) > 1:
            drain_inst.ins.sync_info.on_wait = waits[:1]
            for w in waits[1:]:
                nop = self.nc.sync.nop()
                nop.ins.sync_info = mybir.SyncInfo(on_wait=[w], on_update=[])
        self.nc.all_engine_barrier()
        assert self.sems is not None
        popped = self.nc._tile_sem_poison_stack.pop()
        assert popped is self._sem_poison
        self.nc.clear_and_free_semaphores(list(self.sems.allocated().values()))
        self.nc.all_engine_barrier()

    TileContext._drain_and_barrier = _drain_and_barrier
    TileContext._drain_split_patched = True


def _split_multiwait_instructions(nc):
    """This container's walrus rejects >1 sync wait per instruction; hoist
    extras onto same-engine NOPs placed right before the instruction."""
    from concourse import mybir

    n_split = 0
    for fn in nc.m.functions:
        for bb in fn.blocks:
            out = []
            for inst in bb.instructions:
                si = inst.sync_info
                waits = list(si.on_wait) if si is not None else []
                if len(waits) > 1:
                    for w in waits[:-1]:
                        n_split += 1
                        out.append(
                            mybir.InstNoOp(
                                name=f"waitsplit-{n_split}-{inst.name}",
                                engine=inst.engine,
                                bass_nofuse=True,
                                sync_info=mybir.SyncInfo(on_wait=[w], on_update=[]),
                            )
                        )
                    si.on_wait = waits[-1:]
                out.append(inst)
            if n_split:
                bb.instructions = out


# ---------------------------------------------------------------- builder
_NC_CACHE = {}


def build_nc(use_bq=False, use_bk=False, use_bv=False, use_bp=False, live=None):
    if live is None:
        live = tuple(tuple(True for _ in range(2)) for _ in range(8))
    key = (use_bq, use_bk, use_bv, use_bp, live)
    if key in _NC_CACHE:
        return _NC_CACHE[key]
    _install_ntff_hook()
    _patch_tile_drain()
    import concourse.bass as bass
    import concourse.tile as tile
    from concourse import mybir

    dt = mybir.dt
    AF = mybir.ActivationFunctionType

    nc = bass.Bass("TRN2", target_bir_lowering=False, debug=False, num_devices=8)

    qT_d = nc.declare_dram_parameter("qT", (DIM, L), dt.bfloat16, isOutput=False)
    kT_d = nc.declare_dram_parameter("kT", (DIM, S), dt.bfloat16, isOutput=False)
    vT_d = nc.declare_dram_parameter("vT", (DIM, S), dt.bfloat16, isOutput=False)
    wq_d = nc.declare_dram_parameter("wq", (DIM, DIMC), dt.bfloat16, isOutput=False)
    wk_d = nc.declare_dram_parameter("wk", (DIM, DIMC), dt.bfloat16, isOutput=False)
    wv_d = nc.declare_dram_parameter("wv", (DIM, DIMC), dt.bfloat16, isOutput=False)
    wp_d = nc.declare_dram_parameter("wp", (DIMC, DIM), dt.bfloat16, isOutput=False)
    pb_d = nc.declare_dram_parameter("pbT", (4 * S, 2 * L), dt.bfloat16, isOutput=False)
    bq_d = nc.declare_dram_parameter("bq", (1, DIMC), dt.float32, isOutput=False)
    bk_d = nc.declare_dram_parameter("bk", (1, DIMC), dt.float32, isOutput=False)
    bv_d = nc.declare_dram_parameter("bv", (1, DIMC), dt.float32, isOutput=False)
    bp_d = nc.declare_dram_parameter("bp", (128, 8), dt.float32, isOutput=False)
    out_d = nc.declare_dram_parameter("out", (DIM, L), dt.float32, isOutput=True)

    with tile.TileContext(nc) as tc:
        with (
            tc.tile_pool(name="consts", bufs=1) as consts,
            tc.tile_pool(name="w", bufs=1) as wpool,
            tc.tile_pool(name="heads", bufs=1) as heads,
            tc.tile_pool(name="stage", bufs=1) as stage,
            tc.tile_pool(name="ostage", bufs=4) as ostage,
            tc.tile_pool(name="pb", bufs=2) as pbp,
            tc.tile_pool(name="attn", bufs=6) as attnp,
        ):
            ones_t = consts.tile([128, 64], dt.bfloat16)
            nc.gpsimd.memset(ones_t[:], 1.0)
            if use_bq:
                bq_t = consts.tile([1, DIMC], dt.float32)
                nc.sync.dma_start(bq_t[:], bq_d[:])
            if use_bk:
                bk_t = consts.tile([1, DIMC], dt.float32)
                nc.sync.dma_start(bk_t[:], bk_d[:])
            if use_bv:
                bv_t = consts.tile([1, DIMC], dt.float32)
                nc.sync.dma_start(bv_t[:], bv_d[:])
                ones_f = consts.tile([1, 128], dt.float32)
                nc.gpsimd.memset(ones_f[:], 1.0)
            if use_bq or use_bk:
                ones_r = consts.tile([1, 512], dt.float32)
                nc.gpsimd.memset(ones_r[:], 1.0)

            # big consolidated weight tiles: w*[:, dt*512 + p*128 + ...] = W[dt*128+p-row, col]
            wq_t = wpool.tile([128, 8 * DIMC], dt.bfloat16, name="wqb", tag="wqb")
            wk_t = wpool.tile([128, 8 * DIMC], dt.bfloat16, name="wkb", tag="wkb")
            wv_t = wpool.tile([128, 8 * DIMC], dt.bfloat16, name="wvb", tag="wvb")
            wp_t = wpool.tile([128, 4 * DIM], dt.bfloat16, name="wpb", tag="wpb")

            qh_t = [heads.tile([128, L], dt.bfloat16, name=f"qh{i}", tag=f"qh{i}") for i in range(4)]
            kh_t = [heads.tile([128, S], dt.bfloat16, name=f"kh{i}", tag=f"kh{i}") for i in range(4)]
            vh_t = [heads.tile([128, NHC * 65], dt.bfloat16, name=f"vh{i}", tag=f"vh{i}") for i in range(8)]
            oT_t = [heads.tile([128, L], dt.bfloat16, name=f"oT{i}", tag=f"oT{i}") for i in range(4)]

            def load_big(tile_ap, dram, rows, cols, chunks=1):
                # tile[:, a*cols + c] = dram[a*128 + p, c]
                n_a = rows // 128
                a_per = n_a // chunks
                for ch in range(chunks):
                    nc.sync.dma_start(
                        tile_ap[:, ch * a_per * cols : (ch + 1) * a_per * cols]
                        .rearrange("p (a c) -> p a c", c=cols),
                        dram[ch * a_per * 128 : (ch + 1) * a_per * 128, :]
                        .rearrange("(a p) c -> p a c", p=128),
                    )

            # ======= unified pipeline: one flat PSUM layout (8 banks total) =======
            # projps 2x[128,512] + scps 1x[128,1024] + pvps 3x[65,512] + prps 1
            with (
                tc.tile_pool(name="xT", bufs=1) as xTp,
                tc.tile_pool(name="projps", bufs=2, space="PSUM") as pps,
                tc.tile_pool(name="scps", bufs=1, space="PSUM") as scps,
                tc.tile_pool(name="pvps", bufs=3, space="PSUM") as pvps,
                tc.tile_pool(name="prps", bufs=1, space="PSUM") as prps,
            ):
                # input DMA, dtile-interleaved so accumulation starts on chunk 0
                xq, xk, xv = [], [], []
                for x_l, nm, w_t, w_dram, x_dram in (
                    (xq, "xq", wq_t, wq_d, qT_d),
                    (xk, "xk", wk_t, wk_d, kT_d),
                    (xv, "xv", wv_t, wv_d, vT_d),
                ):
                    for dtile in range(8):
                        nc.sync.dma_start(
                            w_t[:, dtile * 512 : (dtile + 1) * 512],
                            w_dram[dtile * 128 : (dtile + 1) * 128, :],
                        )
                        x_t = xTp.tile(
                            [128, 1024], dt.bfloat16, name=f"{nm}{dtile}", tag=f"{nm}{dtile}"
                        )
                        nc.sync.dma_start(x_t[:], x_dram[dtile * 128 : (dtile + 1) * 128, :])
                        x_l.append(x_t)

                def emit_qk_group(p, kind, lc):
                    lcs = slice(lc * 512, (lc + 1) * 512)
                    w_t, x_l, use_b, dst = (
                        (wq_t, xq, use_bq, qh_t)
                        if kind == "q"
                        else (wk_t, xk, use_bk, kh_t)
                    )
                    ps = pps.tile([128, 512], dt.float32, name=f"ps{kind}{p}_{lc}", tag="ps")
                    for dtile in range(8):
                        nc.tensor.matmul(
                            ps[:],
                            w_t[:, dtile * 512 + p * 128 : dtile * 512 + (p + 1) * 128],
                            x_l[dtile][:, lcs],
                            start=(dtile == 0),
                            stop=(dtile == 7) and not use_b,
                        )
                    if use_b:
                        b_t = bq_t if kind == "q" else bk_t
                        nc.tensor.matmul(
                            ps[:],
                            b_t[0:1, p * 128 : (p + 1) * 128],
                            ones_r[0:1, 0:512],
                            start=False, stop=True,
                        )
                    nc.vector.tensor_copy(dst[p][:, lcs], ps[:])

                def emit_v_block(st):
                    ps = pps.tile([128, 512], dt.float32, name=f"psv{st}", tag="ps")
                    for dtile in range(8):
                        nc.tensor.matmul(
                            ps[:],
                            xv[dtile][:, st * 128 : (st + 1) * 128],
                            wv_t[:, dtile * 512 : (dtile + 1) * 512],
                            start=(dtile == 0),
                            stop=(dtile == 7) and not use_bv,
                        )
                    if use_bv:
                        nc.tensor.matmul(
                            ps[:], ones_f[0:1, 0:128], bv_t[:], start=False, stop=True
                        )
                    nc.gpsimd.memset(vh_t[st][:], 1.0)
                    nc.scalar.copy(
                        vh_t[st].rearrange("p (h x) -> p h x", x=65)[:, :, 0:64],
                        ps.rearrange("p (h x) -> p h x", x=64),
                    )

                # head-pair 0 QK up front; everything else rides the unit loop
                for kind, lc in (("q", 0), ("k", 0), ("q", 1), ("k", 1)):
                    emit_qk_group(0, kind, lc)

                load_big(wp_t, wp_d, DIMC, DIM)
                bp_t = consts.tile([128, 8], dt.float32)
                if use_bp:
                    nc.sync.dma_start(bp_t[:], bp_d[:])

                LAG = 2
                NLAG = 1
                live_sts = {
                    lc: [st for st in range(8) if live[st][lc]] for lc in range(2)
                }
                first_live = {lc: live_sts[lc][0] for lc in range(2)}
                last_live = {lc: live_sts[lc][-1] for lc in range(2)}
                units = [
                    (p, st, lc)
                    for p in range(4)
                    for st in range(8)
                    for lc in range(2)
                    if live[st][lc]
                ]
                ats = {}
                pos = {}
                pbs = {}

                first_unit_of_p = {}
                for idx, (p, st, lc) in enumerate(units):
                    first_unit_of_p.setdefault(p, idx)
                pair_ps = sorted(first_unit_of_p)
                n_in_pair = {
                    p: (first_unit_of_p[pair_ps[k + 1]] if k + 1 < len(pair_ps) else len(units))
                    - first_unit_of_p[p]
                    for k, p in enumerate(pair_ps)
                }

                def load_pb(p):
                    # only live (key-chunk, query-half) blocks are ever read;
                    # cols are (st, lc, j, 512) so a (st, lc) block is contiguous
                    pb_t = pbp.tile([128, 16 * L], dt.bfloat16, name=f"pb{p}", tag="pb")
                    for a in range(8):
                        lcs_live = [lc2 for lc2 in range(2) if live[a][lc2]]
                        if not lcs_live:
                            continue
                        c0 = lcs_live[0] * 1024
                        c1 = (lcs_live[-1] + 1) * 1024
                        nc.sync.dma_start(
                            pb_t[:, a * 2048 + c0 : a * 2048 + c1],
                            pb_d[p * S + a * 128 : p * S + (a + 1) * 128, c0:c1],
                        )
                    pbs[p] = pb_t

                load_pb(pair_ps[0])

                def emit_scores(i):
                    p, st, lc = units[i]
                    lcs = slice(lc * 512, (lc + 1) * 512)
                    if i == first_unit_of_p[p]:
                        k_next = pair_ps.index(p) + 1
                        if k_next < len(pair_ps):
                            load_pb(pair_ps[k_next])
                        for j in range(2):
                            h = 2 * p + j
                            for k in range(2):
                                pos[(h, k)] = pvps.tile(
                                    [65, 512], dt.float32, name=f"po{h}_{k}", tag="pv"
                                )
                    ps = scps.tile([128, 1024], dt.float32, name=f"sc{p}_{st}_{lc}", tag="sc")
                    for j in range(2):
                        jj = j * 64
                        nc.tensor.matmul(
                            ps[:, jj * 8 : jj * 8 + 512],
                            kh_t[p][jj : jj + 64, st * 128 : (st + 1) * 128],
                            qh_t[p][jj : jj + 64, lcs],
                            start=True, stop=True,
                            tile_position=(jj, 0),
                        )
                    at = attnp.tile([128, 1024], dt.bfloat16, name=f"at{p}_{st}_{lc}", tag="attn")
                    nc.scalar.activation(at[:], ps[:], AF.Exp)
                    nc.vector.tensor_mul(
                        at[:], at[:],
                        pbs[p][:, st * 2048 + lc * 1024 : st * 2048 + (lc + 1) * 1024],
                    )
                    ats[(p, st, lc)] = at

                def emit_pv(i):
                    p, st, lc = units[i]
                    at = ats.pop((p, st, lc))
                    for j in range(2):
                        h = 2 * p + j
                        nc.tensor.matmul(
                            pos[(h, lc)][:],
                            vh_t[st][:, h * 65 : h * 65 + 65],
                            at[:, j * 512 : (j + 1) * 512],
                            start=(st == first_live[lc]),
                            stop=(st == last_live[lc]),
                        )

                def emit_norm(p, lc):
                    lcs = slice(lc * 512, (lc + 1) * 512)
                    for j in range(2):
                        h = 2 * p + j
                        po = pos.pop((h, lc))
                        rec_bf = stage.tile(
                            [65, 512], dt.bfloat16, name=f"recbf{h}_{lc}", tag="recbf"
                        )
                        lnr = stage.tile([65, 512], dt.float32, name=f"lnr{h}_{lc}", tag="lnr")
                        nc.scalar.activation(lnr[64:65, :], po[64:65, :], AF.Ln)
                        nc.scalar.activation(
                            rec_bf[64:65, :], lnr[64:65, :], AF.Exp, scale=-1.0
                        )
                        pr = prps.tile([64, 512], dt.float32, name=f"pr{h}_{lc}", tag="pr")
                        nc.tensor.matmul(
                            pr[:],
                            ones_t[64:65, 0:64],
                            rec_bf[64:65, :],
                            start=True, stop=True,
                        )
                        pr_sb = stage.tile(
                            [64, 512], dt.float32, name=f"prsb{h}_{lc}", tag="prsb"
                        )
                        nc.vector.tensor_copy(pr_sb[:], pr[:])
                        nc.vector.tensor_mul(
                            oT_t[p][j * 64 : (j + 1) * 64, lcs], po[0:64, :], pr_sb[:]
                        )

                # PE filler work interleaved into the unit stream: V blocks and
                # next pair's QK groups run in attention-phase PE gaps
                filler = {}
                p0 = pair_ps[0]
                for st in range(8):
                    filler.setdefault(
                        first_unit_of_p[p0] + min(st, n_in_pair[p0] - 1), []
                    ).append(lambda st=st: emit_v_block(st))
                for k, p in enumerate(pair_ps[1:]):
                    src = pair_ps[k]
                    n_src = n_in_pair[src]
                    offs = [8, 9, 10, 11] if k == 0 else [0, 2, 4, 6]
                    for (kind, lc2), o in zip(
                        (("q", 0), ("k", 0), ("q", 1), ("k", 1)), offs
                    ):
                        filler.setdefault(
                            first_unit_of_p[src] + min(o, n_src - 1), []
                        ).append(
                            lambda p=p, kind=kind, lc2=lc2: emit_qk_group(p, kind, lc2)
                        )

                norm_due = {}
                n_units = len(units)
                for i in range(n_units + LAG + NLAG + 4):
                    if i in norm_due:
                        for p, lc in norm_due.pop(i):
                            emit_norm(p, lc)
                    if i < n_units:
                        emit_scores(i)
                    ipv = i - LAG
                    if 0 <= ipv < n_units:
                        emit_pv(ipv)
                        p, st, lc = units[ipv]
                        if st == last_live[lc]:
                            norm_due.setdefault(i + NLAG, []).append((p, lc))
                    for fn in filler.pop(i, []):
                        fn()
                assert not norm_due and not pos and not ats and not filler

                # ========== output projection (lc-outer so lc0 can start early) ==========
                for lc in range(2):
                    lcs = slice(lc * 512, (lc + 1) * 512)
                    for ot in range(8):
                        pf = pps.tile([128, 512], dt.float32, name=f"pf{ot}_{lc}", tag="ps")
                        for p4 in range(4):
                            nc.tensor.matmul(
                                pf[:],
                                wp_t[:, p4 * 1024 + ot * 128 : p4 * 1024 + (ot + 1) * 128],
                                oT_t[p4][:, lcs],
                                start=(p4 == 0),
                                stop=(p4 == 3),
                            )
                        f_sb = ostage.tile([128, 512], dt.float32, name=f"fsb{ot}_{lc}", tag="fsb")
                        if use_bp:
                            nc.scalar.activation(
                                f_sb[:], pf[:], AF.Identity, bias=bp_t[:, ot : ot + 1]
                            )
                        else:
                            nc.scalar.copy(f_sb[:], pf[:])
                        nc.sync.dma_start(
                            out_d[ot * 128 : (ot + 1) * 128, lcs], f_sb[:]
                        )

    _split_multiwait_instructions(nc)
    _NC_CACHE[key] = nc
    return nc


# ---------------------------------------------------------------- host side
def prep_inputs(inputs):
    """Shard + lay out the full inputs into 8 per-core input maps."""
    q = np.asarray(inputs["q"], np.float32)
    k = np.asarray(inputs["k"], np.float32)
    v = np.asarray(inputs["v"], np.float32)
    attn_mask = np.asarray(inputs["attn_mask"], bool)
    pos_bias = np.asarray(inputs["pos_bias"], np.float32)
    Wq = np.asarray(inputs["Wq"], np.float32)
    Wk = np.asarray(inputs["Wk"], np.float32)
    Wv = np.asarray(inputs["Wv"], np.float32)
    Wp = np.asarray(inputs["Wp"], np.float32)
    bq = np.asarray(inputs["bq"], np.float32)
    bk = np.asarray(inputs["bk"], np.float32)
    bv = np.asarray(inputs["bv"], np.float32)
    bp = np.asarray(inputs["bp"], np.float32)
    is_causal = int(np.asarray(inputs["is_causal"]))

    # effective mask: causal + row-any fix (matches the reference exactly)
    mask = attn_mask
    if is_causal:
        causal = np.tril(np.ones((L, L), bool))
        causal = np.pad(causal, ((0, 0), (S - L, 0)), constant_values=True)
        mask = mask & causal[None]
    row_any = mask.any(axis=-1, keepdims=True)
    mask = np.where(row_any, mask, True)  # (B, L, S)

    in_maps = []
    for core in range(8):
        b, hh = core // 2, core % 2
        c0 = hh * DIMC
        h0 = hh * NHC
        wq_c = (Wq[:, c0 : c0 + DIMC] * SCALE).astype(bf16)
        wk_c = Wk[:, c0 : c0 + DIMC].astype(bf16)
        wv_c = Wv[:, c0 : c0 + DIMC].astype(bf16)
        wp_c = Wp[c0 : c0 + DIMC, :].astype(bf16)
        # M = mask * exp(pos_bias), laid out per head-pair as
        # pbT[p*S + key, lc*1024 + j*512 + qc] = M[h=2p+j, query=lc*512+qc, key]
        pbT = np.empty((4 * S, 2 * L), bf16)
        mb = mask[b]  # (L, S)
        for pp in range(4):
            for j in range(2):
                E = np.exp(pos_bias[b, h0 + 2 * pp + j])  # (L, S)
                E *= mb
                for lc in range(2):
                    pbT[
                        pp * S : (pp + 1) * S,
                        lc * 1024 + j * 512 : lc * 1024 + (j + 1) * 512,
                    ] = E[lc * 512 : (lc + 1) * 512, :].T.astype(bf16)
        in_maps.append(
            dict(
                qT=q[b].T.astype(bf16),
                kT=k[b].T.astype(bf16),
                vT=v[b].T.astype(bf16),
                wq=np.ascontiguousarray(wq_c),
                wk=np.ascontiguousarray(wk_c),
                wv=np.ascontiguousarray(wv_c),
                wp=np.ascontiguousarray(wp_c),
                pbT=pbT,
                bq=np.ascontiguousarray((bq[c0 : c0 + DIMC] * SCALE)[None, :]),
                bk=np.ascontiguousarray(bk[c0 : c0 + DIMC][None, :]),
                bv=np.ascontiguousarray(bv[c0 : c0 + DIMC][None, :]),
                bp=(
                    np.ascontiguousarray(bp.reshape(8, 128).T)
                    if hh == 0
                    else np.zeros((128, 8), np.float32)
                ),
            )
        )
    # per-(S-tile, L-chunk) liveness: union of the effective mask over batches.
    # A tile that is all-masked for every batch contributes exactly zero
    # (mask multiply) so the shared SPMD program can skip it entirely.
    mt = mask.any(axis=0)  # (L, S) union over batches
    live = tuple(
        tuple(
            bool(mt[lc * 512 : (lc + 1) * 512, st * 128 : (st + 1) * 128].any())
            for lc in range(2)
        )
        for st in range(8)
    )
    return in_maps, live


def kernel(**inputs):
    global LAST_EXEC_NS
    from concourse.bass_utils import run_bass_kernel_spmd

    in_maps, live = prep_inputs(inputs)
    nc = build_nc(
        use_bq=bool(np.any(np.asarray(inputs["bq"]))),
        use_bk=bool(np.any(np.asarray(inputs["bk"]))),
        use_bv=bool(np.any(np.asarray(inputs["bv"]))),
        use_bp=bool(np.any(np.asarray(inputs["bp"]))),
        live=live,
    )
    kwargs = {}
    if TRACE and TRACE_DIR:
        kwargs["tmpdir"] = TRACE_DIR
    res = run_bass_kernel_spmd(
        nc, in_maps, core_ids=list(range(8)), trace=TRACE, **kwargs
    )
    LAST_EXEC_NS = res.exec_time_ns
    outs = res.results
    out = np.empty((B, L, DIM), np.float32)
    for b in range(B):
        out[b] = (outs[2 * b]["out"] + outs[2 * b + 1]["out"]).T
    return out



# revision 55
# speedup vs baseline: 1.1888x; 1.0357x over previous
"""Trainium2 Bass kernel for nn_AttentionBlock (B=4, L=S=1024, DIM=1024, NH=16).

Sharding: 8 cores = (batch b = core//2) x (head-half hh = core%2, 8 heads each).
Each core computes its batch's QKV projections restricted to its 512 feature
columns, attention for its 8 heads, and a partial output projection
(Wp row-slice); the host sums the two partials per batch.

Device layout is fully transposed ("T" = features/S on partitions) so no
on-device transposes are needed:
  qhT/khT (feat, L|S) from  Wslice.T @ xT ;  scoresT (S, L) = khT.T-slice @ qhT
  pos_bias+mask enter multiplicatively post-exp as M = mask*exp(pos_bias)
  precomputed on host (exp(s+b) = exp(s)*exp(b)); host pre-merges
  causal+row-fix into the mask. Softmax denominators ride a ones-column
  appended to V; normalization happens on the small (64, L) PV output via a
  rank-1 PE broadcast of 1/denom.

Single software-pipelined schedule (one flat 8-bank PSUM layout) so the tile
list-scheduler can overlap everything: QK proj for pair 0 runs up front, then
the attention unit loop (scores pair in concurrent 64-row PE groups -> merged
[128,1024] exp -> M-multiply -> PV) carries V-projection blocks and the next
pair's QK groups as PE filler; causally dead (key-chunk, query-half) tiles are
skipped in both compute and pb DMA. Output projection is lc-outer so its first
half overlaps the final pair's tail. Compute dtype bf16 (f32 PSUM
accumulation), f32 partial outputs.
"""
import contextlib
import ctypes
import sys
import types

import numpy as np
import ml_dtypes

bf16 = ml_dtypes.bfloat16

B, L, S, DIM, NH, DH = 4, 1024, 1024, 1024, 16, 64
NHC = 8           # heads per core
DIMC = 512        # feature columns per core
SCALE = 1.0 / np.sqrt(DH).astype(np.float32)

TRACE = False          # test.py flips this for profiling runs
TRACE_DIR = None
LAST_EXEC_NS = None


# ---------------------------------------------------------------- env setup
def _install_ntff_hook():
    if "antenv.axon_hooks" in sys.modules:
        return
    try:
        lib = ctypes.CDLL("/opt/axon/libaxon_pjrt.so")
        lib.axon_start_nrt_profile.argtypes = [
            ctypes.POINTER(ctypes.c_int64),
            ctypes.c_size_t,
        ]
        lib.axon_start_nrt_profile.restype = ctypes.c_int64
        lib.axon_stop_nrt_profile.argtypes = [ctypes.c_char_p]
        lib.axon_stop_nrt_profile.restype = ctypes.c_int64
    except OSError:
        return

    @contextlib.contextmanager
    def _hook(output_dir, device_ids):
        import jax

        jax.devices()
        if device_ids:
            ids = (ctypes.c_int64 * len(device_ids))(*device_ids)
            rc = lib.axon_start_nrt_profile(ids, len(device_ids))
        else:
            rc = lib.axon_start_nrt_profile(None, 0)
        if rc != 0:
            raise RuntimeError(f"axon_start_nrt_profile rc={rc}")
        try:
            yield
        finally:
            n = lib.axon_stop_nrt_profile(str(output_dir).encode())
            print(f"profile: {n} file(s) written to {output_dir}")

    mod = types.ModuleType("antenv.axon_hooks")
    mod.get_axon_ntff_profile_hook = lambda: _hook
    mod.set_axon_ntff_profile_hook = lambda h: None
    sys.modules["antenv.axon_hooks"] = mod


def _patch_tile_drain():
    from concourse import mybir
    from concourse.tile import TileContext, ScopedClock

    if getattr(TileContext, "_drain_split_patched", False):
        return

    def _drain_and_barrier(self, tick_clock, wait_clock):
        drain_inst = self.nc.sync.drain()
        wait_clock.add_sem_waits(
            drain_inst.ins, ScopedClock({None: tick_clock.global_clock})
        )
        waits = list(drain_inst.ins.sync_info.on_wait)
        if len(waits) > 1:
            drain_inst.ins.sync_info.on_wait = waits[:1]
            for w in waits[1:]:
                nop = self.nc.sync.nop()
                nop.ins.sync_info = mybir.SyncInfo(on_wait=[w], on_update=[])
        self.nc.all_engine_barrier()
        assert self.sems is not None
        popped = self.nc._tile_sem_poison_stack.pop()
        assert popped is self._sem_poison
        self.nc.clear_and_free_semaphores(list(self.sems.allocated().values()))
        self.nc.all_engine_barrier()

    TileContext._drain_and_barrier = _drain_and_barrier
    TileContext._drain_split_patched = True


def _split_multiwait_instructions(nc):
    """This container's walrus rejects >1 sync wait per instruction; hoist
    extras onto same-engine NOPs placed right before the instruction."""
    from concourse import mybir

    n_split = 0
    for fn in nc.m.functions:
        for bb in fn.blocks:
            out = []
            for inst in bb.instructions:
                si = inst.sync_info
                waits = list(si.on_wait) if si is not None else []
                if len(waits) > 1:
                    for w in waits[:-1]:
                        n_split += 1
                        out.append(
                            mybir.InstNoOp(
                                name=f"waitsplit-{n_split}-{inst.name}",
                                engine=inst.engine,
                                bass_nofuse=True,
                                sync_info=mybir.SyncInfo(on_wait=[w], on_update=[]),
                            )
                        )
                    si.on_wait = waits[-1:]
                out.append(inst)
            if n_split:
                bb.instructions = out


# ---------------------------------------------------------------- builder
_NC_CACHE = {}


def build_nc(use_bq=False, use_bk=False, use_bv=False, use_bp=False, live=None):
    if live is None:
        live = tuple(tuple(True for _ in range(2)) for _ in range(8))
    key = (use_bq, use_bk, use_bv, use_bp, live)
    if key in _NC_CACHE:
        return _NC_CACHE[key]
    _install_ntff_hook()
    _patch_tile_drain()
    import concourse.bass as bass
    import concourse.tile as tile
    from concourse import mybir

    dt = mybir.dt
    AF = mybir.ActivationFunctionType

    nc = bass.Bass("TRN2", target_bir_lowering=False, debug=False, num_devices=8)

    qT_d = nc.declare_dram_parameter("qT", (DIM, L), dt.bfloat16, isOutput=False)
    kT_d = nc.declare_dram_parameter("kT", (DIM, S), dt.bfloat16, isOutput=False)
    vT_d = nc.declare_dram_parameter("vT", (DIM, S), dt.bfloat16, isOutput=False)
    wq_d = nc.declare_dram_parameter("wq", (DIM, DIMC), dt.bfloat16, isOutput=False)
    wk_d = nc.declare_dram_parameter("wk", (DIM, DIMC), dt.bfloat16, isOutput=False)
    wv_d = nc.declare_dram_parameter("wv", (DIM, DIMC), dt.bfloat16, isOutput=False)
    wp_d = nc.declare_dram_parameter("wp", (DIMC, DIM), dt.bfloat16, isOutput=False)
    pb_d = nc.declare_dram_parameter("pbT", (4 * S, 2 * L), dt.bfloat16, isOutput=False)
    bq_d = nc.declare_dram_parameter("bq", (1, DIMC), dt.float32, isOutput=False)
    bk_d = nc.declare_dram_parameter("bk", (1, DIMC), dt.float32, isOutput=False)
    bv_d = nc.declare_dram_parameter("bv", (1, DIMC), dt.float32, isOutput=False)
    bp_d = nc.declare_dram_parameter("bp", (128, 8), dt.float32, isOutput=False)
    out_d = nc.declare_dram_parameter("out", (DIM, L), dt.float32, isOutput=True)

    with tile.TileContext(nc) as tc:
        with (
            tc.tile_pool(name="consts", bufs=1) as consts,
            tc.tile_pool(name="w", bufs=1) as wpool,
            tc.tile_pool(name="heads", bufs=1) as heads,
            tc.tile_pool(name="stage", bufs=1) as stage,
            tc.tile_pool(name="ostage", bufs=4) as ostage,
            tc.tile_pool(name="pb", bufs=2) as pbp,
            tc.tile_pool(name="attn", bufs=6) as attnp,
        ):
            ones_t = consts.tile([128, 64], dt.bfloat16)
            nc.gpsimd.memset(ones_t[:], 1.0)
            if use_bq:
                bq_t = consts.tile([1, DIMC], dt.float32)
                nc.sync.dma_start(bq_t[:], bq_d[:])
            if use_bk:
                bk_t = consts.tile([1, DIMC], dt.float32)
                nc.sync.dma_start(bk_t[:], bk_d[:])
            if use_bv:
                bv_t = consts.tile([1, DIMC], dt.float32)
                nc.sync.dma_start(bv_t[:], bv_d[:])
                ones_f = consts.tile([1, 128], dt.float32)
                nc.gpsimd.memset(ones_f[:], 1.0)
            if use_bq or use_bk:
                ones_r = consts.tile([1, 512], dt.float32)
                nc.gpsimd.memset(ones_r[:], 1.0)

            # big consolidated weight tiles: w*[:, dt*512 + p*128 + ...] = W[dt*128+p-row, col]
            wq_t = wpool.tile([128, 8 * DIMC], dt.bfloat16, name="wqb", tag="wqb")
            wk_t = wpool.tile([128, 8 * DIMC], dt.bfloat16, name="wkb", tag="wkb")
            wv_t = wpool.tile([128, 8 * DIMC], dt.bfloat16, name="wvb", tag="wvb")
            wp_t = wpool.tile([128, 4 * DIM], dt.bfloat16, name="wpb", tag="wpb")

            qh_t = [heads.tile([128, L], dt.bfloat16, name=f"qh{i}", tag=f"qh{i}") for i in range(4)]
            kh_t = [heads.tile([128, S], dt.bfloat16, name=f"kh{i}", tag=f"kh{i}") for i in range(4)]
            vh_t = [heads.tile([128, NHC * 65], dt.bfloat16, name=f"vh{i}", tag=f"vh{i}") for i in range(8)]
            oT_t = [heads.tile([128, L], dt.bfloat16, name=f"oT{i}", tag=f"oT{i}") for i in range(4)]

            def load_big(tile_ap, dram, rows, cols, chunks=1):
                # tile[:, a*cols + c] = dram[a*128 + p, c]
                n_a = rows // 128
                a_per = n_a // chunks
                for ch in range(chunks):
                    nc.sync.dma_start(
                        tile_ap[:, ch * a_per * cols : (ch + 1) * a_per * cols]
                        .rearrange("p (a c) -> p a c", c=cols),
                        dram[ch * a_per * 128 : (ch + 1) * a_per * 128, :]
                        .rearrange("(a p) c -> p a c", p=128),
                    )

            # ======= unified pipeline: one flat PSUM layout (8 banks total) =======
            # projps 2x[128,512] + scps 1x[128,1024] + pvps 3x[65,512] + prps 1
            with (
                tc.tile_pool(name="xT", bufs=1) as xTp,
                tc.tile_pool(name="projps", bufs=2, space="PSUM") as pps,
                tc.tile_pool(name="scps", bufs=1, space="PSUM") as scps,
                tc.tile_pool(name="pvps", bufs=3, space="PSUM") as pvps,
                tc.tile_pool(name="prps", bufs=1, space="PSUM") as prps,
            ):
                # input DMA: each dma_start costs ~620ns of sync-queue issue
                # time, so batch into 2 rearranged chunks per tensor (4 dtiles
                # each) instead of per-dtile transfers
                xlists = []
                for nm, w_t, w_dram, x_dram in (
                    ("xq", wq_t, wq_d, qT_d),
                    ("xk", wk_t, wk_d, kT_d),
                    ("xv", wv_t, wv_d, vT_d),
                ):
                    xb_t = xTp.tile([128, 8 * 1024], dt.bfloat16, name=nm, tag=nm)
                    for ch in range(2):
                        nc.sync.dma_start(
                            w_t[:, ch * 2048 : (ch + 1) * 2048]
                            .rearrange("p (a c) -> p a c", c=512),
                            w_dram[ch * 512 : (ch + 1) * 512, :]
                            .rearrange("(a p) c -> p a c", p=128),
                        )
                        nc.sync.dma_start(
                            xb_t[:, ch * 4096 : (ch + 1) * 4096]
                            .rearrange("p (a c) -> p a c", c=1024),
                            x_dram[ch * 512 : (ch + 1) * 512, :]
                            .rearrange("(a p) c -> p a c", p=128),
                        )
                    xlists.append(
                        [xb_t[:, d * 1024 : (d + 1) * 1024] for d in range(8)]
                    )
                xq, xk, xv = xlists

                def emit_qk_group(p, kind, lc):
                    lcs = slice(lc * 512, (lc + 1) * 512)
                    w_t, x_l, use_b, dst = (
                        (wq_t, xq, use_bq, qh_t)
                        if kind == "q"
                        else (wk_t, xk, use_bk, kh_t)
                    )
                    ps = pps.tile([128, 512], dt.float32, name=f"ps{kind}{p}_{lc}", tag="ps")
                    for dtile in range(8):
                        nc.tensor.matmul(
                            ps[:],
                            w_t[:, dtile * 512 + p * 128 : dtile * 512 + (p + 1) * 128],
                            x_l[dtile][:, lcs],
                            start=(dtile == 0),
                            stop=(dtile == 7) and not use_b,
                        )
                    if use_b:
                        b_t = bq_t if kind == "q" else bk_t
                        nc.tensor.matmul(
                            ps[:],
                            b_t[0:1, p * 128 : (p + 1) * 128],
                            ones_r[0:1, 0:512],
                            start=False, stop=True,
                        )
                    nc.vector.tensor_copy(dst[p][:, lcs], ps[:])

                def emit_v_block(st):
                    ps = pps.tile([128, 512], dt.float32, name=f"psv{st}", tag="ps")
                    for dtile in range(8):
                        nc.tensor.matmul(
                            ps[:],
                            xv[dtile][:, st * 128 : (st + 1) * 128],
                            wv_t[:, dtile * 512 : (dtile + 1) * 512],
                            start=(dtile == 0),
                            stop=(dtile == 7) and not use_bv,
                        )
                    if use_bv:
                        nc.tensor.matmul(
                            ps[:], ones_f[0:1, 0:128], bv_t[:], start=False, stop=True
                        )
                    nc.gpsimd.memset(vh_t[st][:], 1.0)
                    nc.scalar.copy(
                        vh_t[st].rearrange("p (h x) -> p h x", x=65)[:, :, 0:64],
                        ps.rearrange("p (h x) -> p h x", x=64),
                    )

                # head-pair 0 QK up front; everything else rides the unit loop
                for kind, lc in (("q", 0), ("k", 0), ("q", 1), ("k", 1)):
                    emit_qk_group(0, kind, lc)

                load_big(wp_t, wp_d, DIMC, DIM)
                bp_t = consts.tile([128, 8], dt.float32)
                if use_bp:
                    nc.sync.dma_start(bp_t[:], bp_d[:])

                LAG = 2
                NLAG = 1
                live_sts = {
                    lc: [st for st in range(8) if live[st][lc]] for lc in range(2)
                }
                first_live = {lc: live_sts[lc][0] for lc in range(2)}
                last_live = {lc: live_sts[lc][-1] for lc in range(2)}
                units = [
                    (p, st, lc)
                    for p in range(4)
                    for st in range(8)
                    for lc in range(2)
                    if live[st][lc]
                ]
                ats = {}
                pos = {}
                pbs = {}

                first_unit_of_p = {}
                for idx, (p, st, lc) in enumerate(units):
                    first_unit_of_p.setdefault(p, idx)
                pair_ps = sorted(first_unit_of_p)
                n_in_pair = {
                    p: (first_unit_of_p[pair_ps[k + 1]] if k + 1 < len(pair_ps) else len(units))
                    - first_unit_of_p[p]
                    for k, p in enumerate(pair_ps)
                }

                def load_pb(p):
                    # only live (key-chunk, query-half) blocks are ever read;
                    # batch contiguous live key-chunk runs into one DMA per
                    # query-half (2 triggers/pair for the causal pattern)
                    pb_t = pbp.tile([128, 16 * L], dt.bfloat16, name=f"pb{p}", tag="pb")
                    for lc2 in range(2):
                        runs = []
                        for a in range(8):
                            if not live[a][lc2]:
                                continue
                            if runs and runs[-1][1] == a:
                                runs[-1][1] = a + 1
                            else:
                                runs.append([a, a + 1])
                        for a0, a1 in runs:
                            nc.sync.dma_start(
                                pb_t.rearrange("p (a w) -> p a w", w=2048)[
                                    :, a0:a1, lc2 * 1024 : (lc2 + 1) * 1024
                                ],
                                pb_d[
                                    p * S + a0 * 128 : p * S + a1 * 128,
                                    lc2 * 1024 : (lc2 + 1) * 1024,
                                ].rearrange("(a pp) c -> pp a c", pp=128),
                            )
                    pbs[p] = pb_t

                load_pb(pair_ps[0])

                def emit_scores(i):
                    p, st, lc = units[i]
                    lcs = slice(lc * 512, (lc + 1) * 512)
                    if i == first_unit_of_p[p]:
                        k_next = pair_ps.index(p) + 1
                        if k_next < len(pair_ps):
                            load_pb(pair_ps[k_next])
                        for j in range(2):
                            h = 2 * p + j
                            for k in range(2):
                                pos[(h, k)] = pvps.tile(
                                    [65, 512], dt.float32, name=f"po{h}_{k}", tag="pv"
                                )
                    ps = scps.tile([128, 1024], dt.float32, name=f"sc{p}_{st}_{lc}", tag="sc")
                    for j in range(2):
                        jj = j * 64
                        nc.tensor.matmul(
                            ps[:, jj * 8 : jj * 8 + 512],
                            kh_t[p][jj : jj + 64, st * 128 : (st + 1) * 128],
                            qh_t[p][jj : jj + 64, lcs],
                            start=True, stop=True,
                            tile_position=(jj, 0),
                        )
                    at = attnp.tile([128, 1024], dt.bfloat16, name=f"at{p}_{st}_{lc}", tag="attn")
                    nc.scalar.activation(at[:], ps[:], AF.Exp)
                    nc.vector.tensor_mul(
                        at[:], at[:],
                        pbs[p][:, st * 2048 + lc * 1024 : st * 2048 + (lc + 1) * 1024],
                    )
                    ats[(p, st, lc)] = at

                def emit_pv(i):
                    p, st, lc = units[i]
                    at = ats.pop((p, st, lc))
                    for j in range(2):
                        h = 2 * p + j
                        nc.tensor.matmul(
                            pos[(h, lc)][:],
                            vh_t[st][:, h * 65 : h * 65 + 65],
                            at[:, j * 512 : (j + 1) * 512],
                            start=(st == first_live[lc]),
                            stop=(st == last_live[lc]),
                        )

                def emit_norm(p, lc):
                    lcs = slice(lc * 512, (lc + 1) * 512)
                    for j in range(2):
                        h = 2 * p + j
                        po = pos.pop((h, lc))
                        rec_bf = stage.tile(
                            [65, 512], dt.bfloat16, name=f"recbf{h}_{lc}", tag="recbf"
                        )
                        lnr = stage.tile([65, 512], dt.float32, name=f"lnr{h}_{lc}", tag="lnr")
                        nc.scalar.activation(lnr[64:65, :], po[64:65, :], AF.Ln)
                        nc.scalar.activation(
                            rec_bf[64:65, :], lnr[64:65, :], AF.Exp, scale=-1.0
                        )
                        pr = prps.tile([64, 512], dt.float32, name=f"pr{h}_{lc}", tag="pr")
                        nc.tensor.matmul(
                            pr[:],
                            ones_t[64:65, 0:64],
                            rec_bf[64:65, :],
                            start=True, stop=True,
                        )
                        pr_sb = stage.tile(
                            [64, 512], dt.float32, name=f"prsb{h}_{lc}", tag="prsb"
                        )
                        nc.vector.tensor_copy(pr_sb[:], pr[:])
                        nc.vector.tensor_mul(
                            oT_t[p][j * 64 : (j + 1) * 64, lcs], po[0:64, :], pr_sb[:]
                        )

                # PE filler work interleaved into the unit stream: V blocks and
                # next pair's QK groups run in attention-phase PE gaps
                filler = {}
                p0 = pair_ps[0]
                for st in range(8):
                    filler.setdefault(
                        first_unit_of_p[p0] + min(st, n_in_pair[p0] - 1), []
                    ).append(lambda st=st: emit_v_block(st))
                for k, p in enumerate(pair_ps[1:]):
                    src = pair_ps[k]
                    n_src = n_in_pair[src]
                    offs = [8, 9, 10, 11] if k == 0 else [0, 2, 4, 6]
                    for (kind, lc2), o in zip(
                        (("q", 0), ("k", 0), ("q", 1), ("k", 1)), offs
                    ):
                        filler.setdefault(
                            first_unit_of_p[src] + min(o, n_src - 1), []
                        ).append(
                            lambda p=p, kind=kind, lc2=lc2: emit_qk_group(p, kind, lc2)
                        )

                norm_due = {}
                n_units = len(units)
                for i in range(n_units + LAG + NLAG + 4):
                    if i in norm_due:
                        for p, lc in norm_due.pop(i):
                            emit_norm(p, lc)
                    if i < n_units:
                        emit_scores(i)
                    ipv = i - LAG
                    if 0 <= ipv < n_units:
                        emit_pv(ipv)
                        p, st, lc = units[ipv]
                        if st == last_live[lc]:
                            norm_due.setdefault(i + NLAG, []).append((p, lc))
                    for fn in filler.pop(i, []):
                        fn()
                assert not norm_due and not pos and not ats and not filler

                # ========== output projection (lc-outer so lc0 can start early) ==========
                for lc in range(2):
                    lcs = slice(lc * 512, (lc + 1) * 512)
                    for ot in range(8):
                        pf = pps.tile([128, 512], dt.float32, name=f"pf{ot}_{lc}", tag="ps")
                        for p4 in range(4):
                            nc.tensor.matmul(
                                pf[:],
                                wp_t[:, p4 * 1024 + ot * 128 : p4 * 1024 + (ot + 1) * 128],
                                oT_t[p4][:, lcs],
                                start=(p4 == 0),
                                stop=(p4 == 3),
                            )
                        f_sb = ostage.tile([128, 512], dt.float32, name=f"fsb{ot}_{lc}", tag="fsb")
                        if use_bp:
                            nc.scalar.activation(
                                f_sb[:], pf[:], AF.Identity, bias=bp_t[:, ot : ot + 1]
                            )
                        else:
                            nc.scalar.copy(f_sb[:], pf[:])
                        nc.sync.dma_start(
                            out_d[ot * 128 : (ot + 1) * 128, lcs], f_sb[:]
                        )

    _split_multiwait_instructions(nc)
    _NC_CACHE[key] = nc
    return nc


# ---------------------------------------------------------------- host side
def prep_inputs(inputs):
    """Shard + lay out the full inputs into 8 per-core input maps."""
    q = np.asarray(inputs["q"], np.float32)
    k = np.asarray(inputs["k"], np.float32)
    v = np.asarray(inputs["v"], np.float32)
    attn_mask = np.asarray(inputs["attn_mask"], bool)
    pos_bias = np.asarray(inputs["pos_bias"], np.float32)
    Wq = np.asarray(inputs["Wq"], np.float32)
    Wk = np.asarray(inputs["Wk"], np.float32)
    Wv = np.asarray(inputs["Wv"], np.float32)
    Wp = np.asarray(inputs["Wp"], np.float32)
    bq = np.asarray(inputs["bq"], np.float32)
    bk = np.asarray(inputs["bk"], np.float32)
    bv = np.asarray(inputs["bv"], np.float32)
    bp = np.asarray(inputs["bp"], np.float32)
    is_causal = int(np.asarray(inputs["is_causal"]))

    # effective mask: causal + row-any fix (matches the reference exactly)
    mask = attn_mask
    if is_causal:
        causal = np.tril(np.ones((L, L), bool))
        causal = np.pad(causal, ((0, 0), (S - L, 0)), constant_values=True)
        mask = mask & causal[None]
    row_any = mask.any(axis=-1, keepdims=True)
    mask = np.where(row_any, mask, True)  # (B, L, S)

    in_maps = []
    for core in range(8):
        b, hh = core // 2, core % 2
        c0 = hh * DIMC
        h0 = hh * NHC
        wq_c = (Wq[:, c0 : c0 + DIMC] * SCALE).astype(bf16)
        wk_c = Wk[:, c0 : c0 + DIMC].astype(bf16)
        wv_c = Wv[:, c0 : c0 + DIMC].astype(bf16)
        wp_c = Wp[c0 : c0 + DIMC, :].astype(bf16)
        # M = mask * exp(pos_bias), laid out per head-pair as
        # pbT[p*S + key, lc*1024 + j*512 + qc] = M[h=2p+j, query=lc*512+qc, key]
        pbT = np.empty((4 * S, 2 * L), bf16)
        mb = mask[b]  # (L, S)
        for pp in range(4):
            for j in range(2):
                E = np.exp(pos_bias[b, h0 + 2 * pp + j])  # (L, S)
                E *= mb
                for lc in range(2):
                    pbT[
                        pp * S : (pp + 1) * S,
                        lc * 1024 + j * 512 : lc * 1024 + (j + 1) * 512,
                    ] = E[lc * 512 : (lc + 1) * 512, :].T.astype(bf16)
        in_maps.append(
            dict(
                qT=q[b].T.astype(bf16),
                kT=k[b].T.astype(bf16),
                vT=v[b].T.astype(bf16),
                wq=np.ascontiguousarray(wq_c),
                wk=np.ascontiguousarray(wk_c),
                wv=np.ascontiguousarray(wv_c),
                wp=np.ascontiguousarray(wp_c),
                pbT=pbT,
                bq=np.ascontiguousarray((bq[c0 : c0 + DIMC] * SCALE)[None, :]),
                bk=np.ascontiguousarray(bk[c0 : c0 + DIMC][None, :]),
                bv=np.ascontiguousarray(bv[c0 : c0 + DIMC][None, :]),
                bp=(
                    np.ascontiguousarray(bp.reshape(8, 128).T)
                    if hh == 0
                    else np.zeros((128, 8), np.float32)
                ),
            )
        )
    # per-(S-tile, L-chunk) liveness: union of the effective mask over batches.
    # A tile that is all-masked for every batch contributes exactly zero
    # (mask multiply) so the shared SPMD program can skip it entirely.
    mt = mask.any(axis=0)  # (L, S) union over batches
    live = tuple(
        tuple(
            bool(mt[lc * 512 : (lc + 1) * 512, st * 128 : (st + 1) * 128].any())
            for lc in range(2)
        )
        for st in range(8)
    )
    return in_maps, live


def kernel(**inputs):
    global LAST_EXEC_NS
    from concourse.bass_utils import run_bass_kernel_spmd

    in_maps, live = prep_inputs(inputs)
    nc = build_nc(
        use_bq=bool(np.any(np.asarray(inputs["bq"]))),
        use_bk=bool(np.any(np.asarray(inputs["bk"]))),
        use_bv=bool(np.any(np.asarray(inputs["bv"]))),
        use_bp=bool(np.any(np.asarray(inputs["bp"]))),
        live=live,
    )
    kwargs = {}
    if TRACE and TRACE_DIR:
        kwargs["tmpdir"] = TRACE_DIR
    res = run_bass_kernel_spmd(
        nc, in_maps, core_ids=list(range(8)), trace=TRACE, **kwargs
    )
    LAST_EXEC_NS = res.exec_time_ns
    outs = res.results
    out = np.empty((B, L, DIM), np.float32)
    for b in range(B):
        out[b] = (outs[2 * b]["out"] + outs[2 * b + 1]["out"]).T
    return out



# revision 56
# speedup vs baseline: 1.2169x; 1.0236x over previous
"""Trainium2 Bass kernel for nn_AttentionBlock (B=4, L=S=1024, DIM=1024, NH=16).

Sharding: 8 cores = (batch b = core//2) x (head-half hh = core%2, 8 heads each).
Each core computes its batch's QKV projections restricted to its 512 feature
columns, attention for its 8 heads, and a partial output projection
(Wp row-slice); the host sums the two partials per batch.

Device layout is fully transposed ("T" = features/S on partitions) so no
on-device transposes are needed:
  qhT/khT (feat, L|S) from  Wslice.T @ xT ;  scoresT (S, L) = khT.T-slice @ qhT
  pos_bias+mask enter multiplicatively post-exp as M = mask*exp(pos_bias)
  precomputed on host (exp(s+b) = exp(s)*exp(b)); host pre-merges
  causal+row-fix into the mask. Softmax denominators ride a ones-column
  appended to V; normalization happens on the small (64, L) PV output via a
  rank-1 PE broadcast of 1/denom.

Single software-pipelined schedule (one flat 8-bank PSUM layout) so the tile
list-scheduler can overlap everything: QK proj for pair 0 runs up front, then
the attention unit loop (scores pair in concurrent 64-row PE groups -> merged
[128,1024] exp -> M-multiply -> PV) carries V-projection blocks and the next
pair's QK groups as PE filler; causally dead (key-chunk, query-half) tiles are
skipped in both compute and pb DMA. Output projection is lc-outer so its first
half overlaps the final pair's tail. Compute dtype bf16 (f32 PSUM
accumulation), f32 partial outputs.
"""
import contextlib
import ctypes
import sys
import types

import numpy as np
import ml_dtypes

bf16 = ml_dtypes.bfloat16

B, L, S, DIM, NH, DH = 4, 1024, 1024, 1024, 16, 64
NHC = 8           # heads per core
DIMC = 512        # feature columns per core
SCALE = 1.0 / np.sqrt(DH).astype(np.float32)

TRACE = False          # test.py flips this for profiling runs
TRACE_DIR = None
LAST_EXEC_NS = None


# ---------------------------------------------------------------- env setup
def _install_ntff_hook():
    if "antenv.axon_hooks" in sys.modules:
        return
    try:
        lib = ctypes.CDLL("/opt/axon/libaxon_pjrt.so")
        lib.axon_start_nrt_profile.argtypes = [
            ctypes.POINTER(ctypes.c_int64),
            ctypes.c_size_t,
        ]
        lib.axon_start_nrt_profile.restype = ctypes.c_int64
        lib.axon_stop_nrt_profile.argtypes = [ctypes.c_char_p]
        lib.axon_stop_nrt_profile.restype = ctypes.c_int64
    except OSError:
        return

    @contextlib.contextmanager
    def _hook(output_dir, device_ids):
        import jax

        jax.devices()
        if device_ids:
            ids = (ctypes.c_int64 * len(device_ids))(*device_ids)
            rc = lib.axon_start_nrt_profile(ids, len(device_ids))
        else:
            rc = lib.axon_start_nrt_profile(None, 0)
        if rc != 0:
            raise RuntimeError(f"axon_start_nrt_profile rc={rc}")
        try:
            yield
        finally:
            n = lib.axon_stop_nrt_profile(str(output_dir).encode())
            print(f"profile: {n} file(s) written to {output_dir}")

    mod = types.ModuleType("antenv.axon_hooks")
    mod.get_axon_ntff_profile_hook = lambda: _hook
    mod.set_axon_ntff_profile_hook = lambda h: None
    sys.modules["antenv.axon_hooks"] = mod


def _patch_tile_drain():
    from concourse import mybir
    from concourse.tile import TileContext, ScopedClock

    if getattr(TileContext, "_drain_split_patched", False):
        return

    def _drain_and_barrier(self, tick_clock, wait_clock):
        drain_inst = self.nc.sync.drain()
        wait_clock.add_sem_waits(
            drain_inst.ins, ScopedClock({None: tick_clock.global_clock})
        )
        waits = list(drain_inst.ins.sync_info.on_wait)
        if len(waits) > 1:
            drain_inst.ins.sync_info.on_wait = waits[:1]
            for w in waits[1:]:
                nop = self.nc.sync.nop()
                nop.ins.sync_info = mybir.SyncInfo(on_wait=[w], on_update=[])
        self.nc.all_engine_barrier()
        assert self.sems is not None
        popped = self.nc._tile_sem_poison_stack.pop()
        assert popped is self._sem_poison
        self.nc.clear_and_free_semaphores(list(self.sems.allocated().values()))
        self.nc.all_engine_barrier()

    TileContext._drain_and_barrier = _drain_and_barrier
    TileContext._drain_split_patched = True


def _split_multiwait_instructions(nc):
    """This container's walrus rejects >1 sync wait per instruction; hoist
    extras onto same-engine NOPs placed right before the instruction."""
    from concourse import mybir

    n_split = 0
    for fn in nc.m.functions:
        for bb in fn.blocks:
            out = []
            for inst in bb.instructions:
                si = inst.sync_info
                waits = list(si.on_wait) if si is not None else []
                if len(waits) > 1:
                    for w in waits[:-1]:
                        n_split += 1
                        out.append(
                            mybir.InstNoOp(
                                name=f"waitsplit-{n_split}-{inst.name}",
                                engine=inst.engine,
                                bass_nofuse=True,
                                sync_info=mybir.SyncInfo(on_wait=[w], on_update=[]),
                            )
                        )
                    si.on_wait = waits[-1:]
                out.append(inst)
            if n_split:
                bb.instructions = out


# ---------------------------------------------------------------- builder
_NC_CACHE = {}


def build_nc(use_bq=False, use_bk=False, use_bv=False, use_bp=False, live=None):
    if live is None:
        live = tuple(tuple(True for _ in range(2)) for _ in range(8))
    key = (use_bq, use_bk, use_bv, use_bp, live)
    if key in _NC_CACHE:
        return _NC_CACHE[key]
    _install_ntff_hook()
    _patch_tile_drain()
    import concourse.bass as bass
    import concourse.tile as tile
    from concourse import mybir

    dt = mybir.dt
    AF = mybir.ActivationFunctionType

    nc = bass.Bass("TRN2", target_bir_lowering=False, debug=False, num_devices=8)

    qT_d = nc.declare_dram_parameter("qT", (DIM, L), dt.bfloat16, isOutput=False)
    kT_d = nc.declare_dram_parameter("kT", (DIM, S), dt.bfloat16, isOutput=False)
    vT_d = nc.declare_dram_parameter("vT", (DIM, S), dt.bfloat16, isOutput=False)
    wq_d = nc.declare_dram_parameter("wq", (DIM, DIMC), dt.bfloat16, isOutput=False)
    wk_d = nc.declare_dram_parameter("wk", (DIM, DIMC), dt.bfloat16, isOutput=False)
    wv_d = nc.declare_dram_parameter("wv", (DIM, DIMC), dt.bfloat16, isOutput=False)
    wp_d = nc.declare_dram_parameter("wp", (DIMC, DIM), dt.bfloat16, isOutput=False)
    pb_d = nc.declare_dram_parameter("pbT", (4 * S, 2 * L), dt.bfloat16, isOutput=False)
    bq_d = nc.declare_dram_parameter("bq", (1, DIMC), dt.float32, isOutput=False)
    bk_d = nc.declare_dram_parameter("bk", (1, DIMC), dt.float32, isOutput=False)
    bv_d = nc.declare_dram_parameter("bv", (1, DIMC), dt.float32, isOutput=False)
    bp_d = nc.declare_dram_parameter("bp", (128, 8), dt.float32, isOutput=False)
    out_d = nc.declare_dram_parameter("out", (DIM, L), dt.float32, isOutput=True)

    with tile.TileContext(nc) as tc:
        with (
            tc.tile_pool(name="consts", bufs=1) as consts,
            tc.tile_pool(name="w", bufs=1) as wpool,
            tc.tile_pool(name="heads", bufs=1) as heads,
            tc.tile_pool(name="stage", bufs=1) as stage,
            tc.tile_pool(name="ostage", bufs=4) as ostage,
            tc.tile_pool(name="pb", bufs=2) as pbp,
            tc.tile_pool(name="attn", bufs=6) as attnp,
        ):
            ones_t = consts.tile([128, 64], dt.bfloat16)
            nc.gpsimd.memset(ones_t[:], 1.0)
            if use_bq:
                bq_t = consts.tile([1, DIMC], dt.float32)
                nc.sync.dma_start(bq_t[:], bq_d[:])
            if use_bk:
                bk_t = consts.tile([1, DIMC], dt.float32)
                nc.sync.dma_start(bk_t[:], bk_d[:])
            if use_bv:
                bv_t = consts.tile([1, DIMC], dt.float32)
                nc.sync.dma_start(bv_t[:], bv_d[:])
                ones_f = consts.tile([1, 128], dt.float32)
                nc.gpsimd.memset(ones_f[:], 1.0)
            if use_bq or use_bk:
                ones_r = consts.tile([1, 512], dt.float32)
                nc.gpsimd.memset(ones_r[:], 1.0)

            # big consolidated weight tiles: w*[:, dt*512 + p*128 + ...] = W[dt*128+p-row, col]
            wq_t = wpool.tile([128, 8 * DIMC], dt.bfloat16, name="wqb", tag="wqb")
            wk_t = wpool.tile([128, 8 * DIMC], dt.bfloat16, name="wkb", tag="wkb")
            wv_t = wpool.tile([128, 8 * DIMC], dt.bfloat16, name="wvb", tag="wvb")
            wp_t = wpool.tile([128, 4 * DIM], dt.bfloat16, name="wpb", tag="wpb")

            qh_t = [heads.tile([128, L], dt.bfloat16, name=f"qh{i}", tag=f"qh{i}") for i in range(4)]
            kh_t = [heads.tile([128, S], dt.bfloat16, name=f"kh{i}", tag=f"kh{i}") for i in range(4)]
            vh_t = [heads.tile([128, NHC * 65], dt.bfloat16, name=f"vh{i}", tag=f"vh{i}") for i in range(8)]
            oT_t = [heads.tile([128, L], dt.bfloat16, name=f"oT{i}", tag=f"oT{i}") for i in range(4)]

            def load_big(tile_ap, dram, rows, cols, chunks=1):
                # tile[:, a*cols + c] = dram[a*128 + p, c]
                n_a = rows // 128
                a_per = n_a // chunks
                for ch in range(chunks):
                    nc.sync.dma_start(
                        tile_ap[:, ch * a_per * cols : (ch + 1) * a_per * cols]
                        .rearrange("p (a c) -> p a c", c=cols),
                        dram[ch * a_per * 128 : (ch + 1) * a_per * 128, :]
                        .rearrange("(a p) c -> p a c", p=128),
                    )

            # ======= unified pipeline: one flat PSUM layout (8 banks total) =======
            # projps 2x[128,512] + scps 1x[128,1024] + pvps 3x[65,512] + prps 1
            with (
                tc.tile_pool(name="xT", bufs=1) as xTp,
                tc.tile_pool(name="projps", bufs=2, space="PSUM") as pps,
                tc.tile_pool(name="scps", bufs=1, space="PSUM") as scps,
                tc.tile_pool(name="pvps", bufs=3, space="PSUM") as pvps,
                tc.tile_pool(name="prps", bufs=1, space="PSUM") as prps,
            ):
                # input DMA: each dma_start costs ~620ns of sync-queue issue
                # time, so batch into 2 rearranged chunks per tensor (4 dtiles
                # each) instead of per-dtile transfers
                xlists = []
                for nm, w_t, w_dram, x_dram in (
                    ("xq", wq_t, wq_d, qT_d),
                    ("xk", wk_t, wk_d, kT_d),
                    ("xv", wv_t, wv_d, vT_d),
                ):
                    xb_t = xTp.tile([128, 8 * 1024], dt.bfloat16, name=nm, tag=nm)
                    for ch in range(4):
                        nc.sync.dma_start(
                            w_t[:, ch * 1024 : (ch + 1) * 1024]
                            .rearrange("p (a c) -> p a c", c=512),
                            w_dram[ch * 256 : (ch + 1) * 256, :]
                            .rearrange("(a p) c -> p a c", p=128),
                        )
                        nc.sync.dma_start(
                            xb_t[:, ch * 2048 : (ch + 1) * 2048]
                            .rearrange("p (a c) -> p a c", c=1024),
                            x_dram[ch * 256 : (ch + 1) * 256, :]
                            .rearrange("(a p) c -> p a c", p=128),
                        )
                    xlists.append(
                        [xb_t[:, d * 1024 : (d + 1) * 1024] for d in range(8)]
                    )
                xq, xk, xv = xlists

                def emit_qk_group(p, kind, lc):
                    lcs = slice(lc * 512, (lc + 1) * 512)
                    w_t, x_l, use_b, dst = (
                        (wq_t, xq, use_bq, qh_t)
                        if kind == "q"
                        else (wk_t, xk, use_bk, kh_t)
                    )
                    ps = pps.tile([128, 512], dt.float32, name=f"ps{kind}{p}_{lc}", tag="ps")
                    for dtile in range(8):
                        nc.tensor.matmul(
                            ps[:],
                            w_t[:, dtile * 512 + p * 128 : dtile * 512 + (p + 1) * 128],
                            x_l[dtile][:, lcs],
                            start=(dtile == 0),
                            stop=(dtile == 7) and not use_b,
                        )
                    if use_b:
                        b_t = bq_t if kind == "q" else bk_t
                        nc.tensor.matmul(
                            ps[:],
                            b_t[0:1, p * 128 : (p + 1) * 128],
                            ones_r[0:1, 0:512],
                            start=False, stop=True,
                        )
                    nc.vector.tensor_copy(dst[p][:, lcs], ps[:])

                def emit_v_block(st):
                    ps = pps.tile([128, 512], dt.float32, name=f"psv{st}", tag="ps")
                    for dtile in range(8):
                        nc.tensor.matmul(
                            ps[:],
                            xv[dtile][:, st * 128 : (st + 1) * 128],
                            wv_t[:, dtile * 512 : (dtile + 1) * 512],
                            start=(dtile == 0),
                            stop=(dtile == 7) and not use_bv,
                        )
                    if use_bv:
                        nc.tensor.matmul(
                            ps[:], ones_f[0:1, 0:128], bv_t[:], start=False, stop=True
                        )
                    nc.gpsimd.memset(vh_t[st][:], 1.0)
                    nc.scalar.copy(
                        vh_t[st].rearrange("p (h x) -> p h x", x=65)[:, :, 0:64],
                        ps.rearrange("p (h x) -> p h x", x=64),
                    )

                # head-pair 0 QK up front; everything else rides the unit loop
                for kind, lc in (("q", 0), ("k", 0), ("q", 1), ("k", 1)):
                    emit_qk_group(0, kind, lc)

                load_big(wp_t, wp_d, DIMC, DIM)
                bp_t = consts.tile([128, 8], dt.float32)
                if use_bp:
                    nc.sync.dma_start(bp_t[:], bp_d[:])

                LAG = 2
                NLAG = 1
                live_sts = {
                    lc: [st for st in range(8) if live[st][lc]] for lc in range(2)
                }
                first_live = {lc: live_sts[lc][0] for lc in range(2)}
                last_live = {lc: live_sts[lc][-1] for lc in range(2)}
                units = [
                    (p, st, lc)
                    for p in range(4)
                    for st in range(8)
                    for lc in range(2)
                    if live[st][lc]
                ]
                ats = {}
                pos = {}
                pbs = {}

                first_unit_of_p = {}
                for idx, (p, st, lc) in enumerate(units):
                    first_unit_of_p.setdefault(p, idx)
                pair_ps = sorted(first_unit_of_p)
                n_in_pair = {
                    p: (first_unit_of_p[pair_ps[k + 1]] if k + 1 < len(pair_ps) else len(units))
                    - first_unit_of_p[p]
                    for k, p in enumerate(pair_ps)
                }

                def load_pb(p):
                    # only live (key-chunk, query-half) blocks are ever read;
                    # batch contiguous live key-chunk runs into one DMA per
                    # query-half (2 triggers/pair for the causal pattern)
                    pb_t = pbp.tile([128, 16 * L], dt.bfloat16, name=f"pb{p}", tag="pb")
                    for lc2 in range(2):
                        runs = []
                        for a in range(8):
                            if not live[a][lc2]:
                                continue
                            if runs and runs[-1][1] == a:
                                runs[-1][1] = a + 1
                            else:
                                runs.append([a, a + 1])
                        for a0, a1 in runs:
                            nc.sync.dma_start(
                                pb_t.rearrange("p (a w) -> p a w", w=2048)[
                                    :, a0:a1, lc2 * 1024 : (lc2 + 1) * 1024
                                ],
                                pb_d[
                                    p * S + a0 * 128 : p * S + a1 * 128,
                                    lc2 * 1024 : (lc2 + 1) * 1024,
                                ].rearrange("(a pp) c -> pp a c", pp=128),
                            )
                    pbs[p] = pb_t

                load_pb(pair_ps[0])

                def emit_scores(i):
                    p, st, lc = units[i]
                    lcs = slice(lc * 512, (lc + 1) * 512)
                    if i == first_unit_of_p[p]:
                        k_next = pair_ps.index(p) + 1
                        if k_next < len(pair_ps):
                            load_pb(pair_ps[k_next])
                        for j in range(2):
                            h = 2 * p + j
                            for k in range(2):
                                pos[(h, k)] = pvps.tile(
                                    [65, 512], dt.float32, name=f"po{h}_{k}", tag="pv"
                                )
                    ps = scps.tile([128, 1024], dt.float32, name=f"sc{p}_{st}_{lc}", tag="sc")
                    for j in range(2):
                        jj = j * 64
                        nc.tensor.matmul(
                            ps[:, jj * 8 : jj * 8 + 512],
                            kh_t[p][jj : jj + 64, st * 128 : (st + 1) * 128],
                            qh_t[p][jj : jj + 64, lcs],
                            start=True, stop=True,
                            tile_position=(jj, 0),
                        )
                    at = attnp.tile([128, 1024], dt.bfloat16, name=f"at{p}_{st}_{lc}", tag="attn")
                    nc.scalar.activation(at[:], ps[:], AF.Exp)
                    nc.vector.tensor_mul(
                        at[:], at[:],
                        pbs[p][:, st * 2048 + lc * 1024 : st * 2048 + (lc + 1) * 1024],
                    )
                    ats[(p, st, lc)] = at

                def emit_pv(i):
                    p, st, lc = units[i]
                    at = ats.pop((p, st, lc))
                    for j in range(2):
                        h = 2 * p + j
                        nc.tensor.matmul(
                            pos[(h, lc)][:],
                            vh_t[st][:, h * 65 : h * 65 + 65],
                            at[:, j * 512 : (j + 1) * 512],
                            start=(st == first_live[lc]),
                            stop=(st == last_live[lc]),
                        )

                def emit_norm(p, lc):
                    lcs = slice(lc * 512, (lc + 1) * 512)
                    for j in range(2):
                        h = 2 * p + j
                        po = pos.pop((h, lc))
                        rec_bf = stage.tile(
                            [65, 512], dt.bfloat16, name=f"recbf{h}_{lc}", tag="recbf"
                        )
                        lnr = stage.tile([65, 512], dt.float32, name=f"lnr{h}_{lc}", tag="lnr")
                        nc.scalar.activation(lnr[64:65, :], po[64:65, :], AF.Ln)
                        nc.scalar.activation(
                            rec_bf[64:65, :], lnr[64:65, :], AF.Exp, scale=-1.0
                        )
                        pr = prps.tile([64, 512], dt.float32, name=f"pr{h}_{lc}", tag="pr")
                        nc.tensor.matmul(
                            pr[:],
                            ones_t[64:65, 0:64],
                            rec_bf[64:65, :],
                            start=True, stop=True,
                        )
                        pr_sb = stage.tile(
                            [64, 512], dt.float32, name=f"prsb{h}_{lc}", tag="prsb"
                        )
                        nc.vector.tensor_copy(pr_sb[:], pr[:])
                        nc.vector.tensor_mul(
                            oT_t[p][j * 64 : (j + 1) * 64, lcs], po[0:64, :], pr_sb[:]
                        )

                # PE filler work interleaved into the unit stream: V blocks and
                # next pair's QK groups run in attention-phase PE gaps
                filler = {}
                p0 = pair_ps[0]
                for st in range(8):
                    filler.setdefault(
                        first_unit_of_p[p0] + min(st, n_in_pair[p0] - 1), []
                    ).append(lambda st=st: emit_v_block(st))
                for k, p in enumerate(pair_ps[1:]):
                    src = pair_ps[k]
                    n_src = n_in_pair[src]
                    offs = [8, 9, 10, 11] if k == 0 else [0, 2, 4, 6]
                    for (kind, lc2), o in zip(
                        (("q", 0), ("k", 0), ("q", 1), ("k", 1)), offs
                    ):
                        filler.setdefault(
                            first_unit_of_p[src] + min(o, n_src - 1), []
                        ).append(
                            lambda p=p, kind=kind, lc2=lc2: emit_qk_group(p, kind, lc2)
                        )

                norm_due = {}
                n_units = len(units)
                for i in range(n_units + LAG + NLAG + 4):
                    if i in norm_due:
                        for p, lc in norm_due.pop(i):
                            emit_norm(p, lc)
                    if i < n_units:
                        emit_scores(i)
                    ipv = i - LAG
                    if 0 <= ipv < n_units:
                        emit_pv(ipv)
                        p, st, lc = units[ipv]
                        if st == last_live[lc]:
                            norm_due.setdefault(i + NLAG, []).append((p, lc))
                    for fn in filler.pop(i, []):
                        fn()
                assert not norm_due and not pos and not ats and not filler

                # ========== output projection (lc-outer so lc0 can start early) ==========
                for lc in range(2):
                    lcs = slice(lc * 512, (lc + 1) * 512)
                    for ot in range(8):
                        pf = pps.tile([128, 512], dt.float32, name=f"pf{ot}_{lc}", tag="ps")
                        for p4 in range(4):
                            nc.tensor.matmul(
                                pf[:],
                                wp_t[:, p4 * 1024 + ot * 128 : p4 * 1024 + (ot + 1) * 128],
                                oT_t[p4][:, lcs],
                                start=(p4 == 0),
                                stop=(p4 == 3),
                            )
                        f_sb = ostage.tile([128, 512], dt.float32, name=f"fsb{ot}_{lc}", tag="fsb")
                        if use_bp:
                            nc.scalar.activation(
                                f_sb[:], pf[:], AF.Identity, bias=bp_t[:, ot : ot + 1]
                            )
                        else:
                            nc.scalar.copy(f_sb[:], pf[:])
                        nc.sync.dma_start(
                            out_d[ot * 128 : (ot + 1) * 128, lcs], f_sb[:]
                        )

    _split_multiwait_instructions(nc)
    _NC_CACHE[key] = nc
    return nc


# ---------------------------------------------------------------- host side
def prep_inputs(inputs):
    """Shard + lay out the full inputs into 8 per-core input maps."""
    q = np.asarray(inputs["q"], np.float32)
    k = np.asarray(inputs["k"], np.float32)
    v = np.asarray(inputs["v"], np.float32)
    attn_mask = np.asarray(inputs["attn_mask"], bool)
    pos_bias = np.asarray(inputs["pos_bias"], np.float32)
    Wq = np.asarray(inputs["Wq"], np.float32)
    Wk = np.asarray(inputs["Wk"], np.float32)
    Wv = np.asarray(inputs["Wv"], np.float32)
    Wp = np.asarray(inputs["Wp"], np.float32)
    bq = np.asarray(inputs["bq"], np.float32)
    bk = np.asarray(inputs["bk"], np.float32)
    bv = np.asarray(inputs["bv"], np.float32)
    bp = np.asarray(inputs["bp"], np.float32)
    is_causal = int(np.asarray(inputs["is_causal"]))

    # effective mask: causal + row-any fix (matches the reference exactly)
    mask = attn_mask
    if is_causal:
        causal = np.tril(np.ones((L, L), bool))
        causal = np.pad(causal, ((0, 0), (S - L, 0)), constant_values=True)
        mask = mask & causal[None]
    row_any = mask.any(axis=-1, keepdims=True)
    mask = np.where(row_any, mask, True)  # (B, L, S)

    in_maps = []
    for core in range(8):
        b, hh = core // 2, core % 2
        c0 = hh * DIMC
        h0 = hh * NHC
        wq_c = (Wq[:, c0 : c0 + DIMC] * SCALE).astype(bf16)
        wk_c = Wk[:, c0 : c0 + DIMC].astype(bf16)
        wv_c = Wv[:, c0 : c0 + DIMC].astype(bf16)
        wp_c = Wp[c0 : c0 + DIMC, :].astype(bf16)
        # M = mask * exp(pos_bias), laid out per head-pair as
        # pbT[p*S + key, lc*1024 + j*512 + qc] = M[h=2p+j, query=lc*512+qc, key]
        pbT = np.empty((4 * S, 2 * L), bf16)
        mb = mask[b]  # (L, S)
        for pp in range(4):
            for j in range(2):
                E = np.exp(pos_bias[b, h0 + 2 * pp + j])  # (L, S)
                E *= mb
                for lc in range(2):
                    pbT[
                        pp * S : (pp + 1) * S,
                        lc * 1024 + j * 512 : lc * 1024 + (j + 1) * 512,
                    ] = E[lc * 512 : (lc + 1) * 512, :].T.astype(bf16)
        in_maps.append(
            dict(
                qT=q[b].T.astype(bf16),
                kT=k[b].T.astype(bf16),
                vT=v[b].T.astype(bf16),
                wq=np.ascontiguousarray(wq_c),
                wk=np.ascontiguousarray(wk_c),
                wv=np.ascontiguousarray(wv_c),
                wp=np.ascontiguousarray(wp_c),
                pbT=pbT,
                bq=np.ascontiguousarray((bq[c0 : c0 + DIMC] * SCALE)[None, :]),
                bk=np.ascontiguousarray(bk[c0 : c0 + DIMC][None, :]),
                bv=np.ascontiguousarray(bv[c0 : c0 + DIMC][None, :]),
                bp=(
                    np.ascontiguousarray(bp.reshape(8, 128).T)
                    if hh == 0
                    else np.zeros((128, 8), np.float32)
                ),
            )
        )
    # per-(S-tile, L-chunk) liveness: union of the effective mask over batches.
    # A tile that is all-masked for every batch contributes exactly zero
    # (mask multiply) so the shared SPMD program can skip it entirely.
    mt = mask.any(axis=0)  # (L, S) union over batches
    live = tuple(
        tuple(
            bool(mt[lc * 512 : (lc + 1) * 512, st * 128 : (st + 1) * 128].any())
            for lc in range(2)
        )
        for st in range(8)
    )
    return in_maps, live


def kernel(**inputs):
    global LAST_EXEC_NS
    from concourse.bass_utils import run_bass_kernel_spmd

    in_maps, live = prep_inputs(inputs)
    nc = build_nc(
        use_bq=bool(np.any(np.asarray(inputs["bq"]))),
        use_bk=bool(np.any(np.asarray(inputs["bk"]))),
        use_bv=bool(np.any(np.asarray(inputs["bv"]))),
        use_bp=bool(np.any(np.asarray(inputs["bp"]))),
        live=live,
    )
    kwargs = {}
    if TRACE and TRACE_DIR:
        kwargs["tmpdir"] = TRACE_DIR
    res = run_bass_kernel_spmd(
        nc, in_maps, core_ids=list(range(8)), trace=TRACE, **kwargs
    )
    LAST_EXEC_NS = res.exec_time_ns
    outs = res.results
    out = np.empty((B, L, DIM), np.float32)
    for b in range(B):
        out[b] = (outs[2 * b]["out"] + outs[2 * b + 1]["out"]).T
    return out

